# revision 30
# baseline (speedup 1.0000x reference)
"""MoE top-1 routing kernel for 8 TRN2 NeuronCores (expert parallelism).

Self-contained: takes full inputs, shards experts across 8 cores, returns the
full output (host sums the 8 disjoint per-expert partials).

v5 design (local-first pipelining, single collective):
- Gating is token-sharded: each core computes fp32 logits for its own 1024
  tokens, then DVE softmax/argmax and local (shard-internal) queue positions
  via one triangular matmul. No counts-AllGather is needed: each shard
  scatters (token_id+1, gate) into per-(shard,expert) regions of a [E*CL,2]
  send buffer at LOCAL positions (CL=512 rows/region, row 0 = count), and one
  32 KB AllToAll delivers region e to core e.
- Latency hiding: tokens of a core's OWN shard choosing its OWN expert (the
  "A set", local pos < 256) are known before the AllToAll; the core gathers
  them from the replicated token buffer and starts the FFN on them (2 chunks
  of 128 slots) while the collective + receive-side compaction for the
  remaining "B set" (8 chunks) is still in flight. Queue order is irrelevant
  to the math: outputs are scattered back by token id, and capacity drops
  (global pos >= C) are applied exactly at output-scatter time using the
  received per-shard counts.
- Receive-side compaction is a computed gather: per-shard counts (embedded at
  region row 0) -> prefix sums on DVE -> per-slot source index + global
  position -> one level of 8B-row indirect gathers, then the usual 2KB-row
  token gathers from the bf16 token buffer.
- FFN in bf16: w2 resident in SBUF (8 MB), w1 streamed once per pass (A then
  B), fused bias+ReLU on the scalar engine, gate-scaled rows scattered into
  the output by token id.
"""
import numpy as np
import ml_dtypes
from contextlib import ExitStack

import concourse.bass as bass
import concourse.tile as tile
from concourse import bacc, mybir
from concourse.bass_utils import run_bass_kernel_spmd

dt = mybir.dt

B, S, M, E, DFF = 4, 2048, 1024, 8, 4096
T = B * S
C = int(1.25 * T / E)          # 1280 capacity per expert
P = 128
MC = M // P                    # 8
DC = DFF // P                  # 32
TSH = T // E                   # 1024 tokens per shard
LT = TSH // P                  # 8
LE = LT * E                    # 64
CL = 512                       # rows per (shard, expert) region (row0=count)
RCV = E * CL                   # 4096
NA = 2                         # local-first chunks (256 slots)
NAS = NA * P
NB = 8                         # remote/compacted chunks (1024 slots)
NBS = NB * P
BIG = 1.5e9

_CACHE = {}


def _build_nc():
    nc = bacc.Bacc("TRN2", target_bir_lowering=False, debug=False)

    xTs = nc.dram_tensor("xTs", [M, TSH], dt.float32, kind="ExternalInput")
    xb = nc.dram_tensor("xb", [T, M], dt.bfloat16, kind="ExternalInput")
    wg = nc.dram_tensor("wg", [M, E], dt.float32, kind="ExternalInput")
    w1p = nc.dram_tensor("w1p", [DC, P, MC, P], dt.bfloat16, kind="ExternalInput")
    w2p = nc.dram_tensor("w2p", [P, DC, M], dt.bfloat16, kind="ExternalInput")
    b1v = nc.dram_tensor("b1v", [DFF], dt.float32, kind="ExternalInput")
    b2b = nc.dram_tensor("b2b", [P, M], dt.float32, kind="ExternalInput")
    cpakd = nc.dram_tensor("cpakd", [P, 408], dt.float32, kind="ExternalInput")
    identb = nc.dram_tensor("identb", [P, P], dt.bfloat16, kind="ExternalInput")
    w64d = nc.dram_tensor("w64d", [LE, LE + E], dt.float32, kind="ExternalInput")
    melocd = nc.dram_tensor("melocd", [P, NA], dt.int32, kind="ExternalInput")
    percd = nc.dram_tensor("percd", [1, 24], dt.float32, kind="ExternalInput")
    outd = nc.dram_tensor("out", [T, M], dt.float32, kind="ExternalOutput")

    igd_loc = nc.dram_tensor("igd_loc", [RCV, 2], dt.float32)
    igd_ps = [nc.dram_tensor(f"igd_p{i}", [RCV, 2], dt.float32)
              for i in range(4)]
    igd_rcv = nc.dram_tensor("igd_rcv", [RCV, 2], dt.float32)
    scrd = nc.dram_tensor("scrd", [1, 32], dt.float32)
    wrm_l = nc.dram_tensor("wrm_l", [8, 2], dt.float32)
    wrm_a = nc.dram_tensor("wrm_a", [64, 2], dt.float32, addr_space="Shared")

    fp32 = dt.float32
    bf16 = dt.bfloat16

    with tile.TileContext(nc) as tc, ExitStack() as ctx:
        sb = ctx.enter_context(tc.tile_pool(name="sb", bufs=1))
        sbx = ctx.enter_context(tc.tile_pool(name="sbx", bufs=6))
        sbw1 = ctx.enter_context(tc.tile_pool(name="sbw1", bufs=4))
        sbg = ctx.enter_context(tc.tile_pool(name="sbg", bufs=4))
        sbst = ctx.enter_context(tc.tile_pool(name="sbst", bufs=2))

        # ---- warmup collective: absorbs CC channel init + start barrier
        wz = sb.tile([8, 2], fp32)
        nc.vector.memset(wz[:], 0.0)
        nc.sync.dma_start(wrm_l[:], wz[:])
        nc.gpsimd.collective_compute(
            "AllGather", mybir.AluOpType.bypass,
            ins=[wrm_l[:]], outs=[wrm_a[:]],
            replica_groups=[list(range(E))])

        # ---- const loads (sync queue; xTs chunks follow in the gating loop)
        wgt = sb.tile([P, MC * E], fp32)
        nc.sync.dma_start(wgt[:], wg[:].rearrange("(mc p) e -> p mc e", p=P))
        cpak = sb.tile([P, 408], fp32)
        nc.sync.dma_start(cpak[:], cpakd[:])
        eit = cpak[:, 0:64]
        trit = cpak[:, 64:192]
        idf = cpak[:, 192:320]
        eclt = cpak[:, 320:384]
        tokp1 = cpak[:, 384:392]
        qiot = cpak[:, 392:400]
        siot = cpak[:, 400:408]
        idb = sb.tile([P, P], bf16)
        nc.sync.dma_start(idb[:], identb[:])
        w64t = sb.tile([LE, LE + E], fp32)
        nc.sync.dma_start(w64t[:], w64d[:])
        meloct = sb.tile([P, NA], dt.int32)
        nc.sync.dma_start(meloct[:], melocd[:])
        percs = sb.tile([1, 24], fp32)
        nc.sync.dma_start(percs[:], percd[:])
        b1t = sb.tile([P, DC], fp32)
        nc.sync.dma_start(b1t[:], b1v[:].rearrange("(d p) -> p d", p=P))
        # PE warm-up: keep the HAM activity window busy before the logits MMs
        wrmj = sb.tile([P, P], bf16)
        nc.vector.memset(wrmj[:], 0.0)
        # scalar queue: b2 broadcast + zero-prefill of the send buffer
        b2t = sb.tile([P, M], fp32)
        nc.scalar.dma_start(b2t[:], b2b[:])
        zpre = sb.tile([P, RCV * 2 // P], fp32)
        nc.vector.memset(zpre[:], 0.0)
        for i in range(4):
            nc.scalar.dma_start(
                igd_ps[i][:].rearrange("(p c) two -> p c two", p=P), zpre[:])

        ones1 = sb.tile([1, P], fp32)
        nc.vector.memset(ones1[:], 1.0)
        onescol = sb.tile([P, 1], fp32)
        nc.vector.memset(onescol[:], 1.0)
        nines = sb.tile([P, LE], fp32)
        nc.vector.memset(nines[:], 9.0)
        huget = sb.tile([P, LE], fp32)
        nc.vector.memset(huget[:], BIG)
        bigA = sb.tile([P, NA], fp32)
        nc.vector.memset(bigA[:], BIG)
        bigB = sb.tile([P, NB], fp32)
        nc.vector.memset(bigB[:], BIG)

        # ---- gating: fp32 logits for my 1024 tokens
        eg_stk = sb.tile([P, LT * 2], fp32)
        lg_stk = sb.tile([P, LE], fp32)
        lgT = sb.tile([8, TSH], fp32)
        with tc.tile_pool(name="psg", bufs=2, space="PSUM") as psg:
            pwrm = psg.tile([P, P], fp32, tag="pwrm")
            for _w in range(24):
                nc.tensor.matmul(pwrm[:], lhsT=wrmj[:], rhs=idb[:],
                                 start=(_w == 0), stop=(_w == 23))
            pl0 = psg.tile([8, 512], fp32, tag="pl0")
            pl1 = psg.tile([8, 512], fp32, tag="pl1")
            for k in range(MC):
                xt0 = sbx.tile([P, 512], fp32, tag="xt")
                nc.sync.dma_start(xt0[:], xTs[k * P:(k + 1) * P, 0:512])
                nc.tensor.matmul(
                    pl0[:], lhsT=wgt[:, k * E:(k + 1) * E], rhs=xt0[:],
                    start=(k == 0), stop=(k == MC - 1))
                xt1 = sbx.tile([P, 512], fp32, tag="xt")
                nc.sync.dma_start(xt1[:], xTs[k * P:(k + 1) * P, 512:1024])
                nc.tensor.matmul(
                    pl1[:], lhsT=wgt[:, k * E:(k + 1) * E], rhs=xt1[:],
                    start=(k == 0), stop=(k == MC - 1))
            nc.vector.tensor_copy(lgT[:, 0:512], pl0[:])
            nc.vector.tensor_copy(lgT[:, 512:1024], pl1[:])
            for ti in range(LT):
                pq = psg.tile([P, E], fp32, tag="pq")
                nc.tensor.transpose(
                    out=pq[:], in_=lgT[:, ti * P:(ti + 1) * P],
                    identity=idf[:8, :8])
                nc.vector.tensor_copy(lg_stk[:, ti * E:(ti + 1) * E], pq[:])
        lg3 = lg_stk[:].rearrange("p (ti e) -> p ti e", e=E)
        mx_stk = sb.tile([P, LT], fp32)
        nc.vector.tensor_reduce(
            out=mx_stk[:], in_=lg3, axis=mybir.AxisListType.X,
            op=mybir.AluOpType.max)
        mxb = mx_stk[:].rearrange("p (ti one) -> p ti one", one=1).to_broadcast([P, LT, E])
        ls = sb.tile([P, LE], fp32)
        nc.vector.tensor_tensor(
            out=ls[:].rearrange("p (ti e) -> p ti e", e=E), in0=lg3, in1=mxb,
            op=mybir.AluOpType.subtract)
        ex = sb.tile([P, LE], fp32)
        nc.scalar.activation(ex[:], ls[:], mybir.ActivationFunctionType.Exp)
        s_stk = sb.tile([P, LT], fp32)
        nc.vector.tensor_reduce(
            out=s_stk[:], in_=ex[:].rearrange("p (ti e) -> p ti e", e=E),
            axis=mybir.AxisListType.X, op=mybir.AluOpType.add)
        nc.vector.reciprocal(
            eg_stk[:].rearrange("p (ti two) -> p ti two", two=2)[:, :, 1:2],
            s_stk[:].rearrange("p (ti one) -> p ti one", one=1))
        oh = sb.tile([P, LE], dt.uint8)
        nc.vector.tensor_tensor(
            out=oh[:].rearrange("p (ti e) -> p ti e", e=E), in0=lg3, in1=mxb,
            op=mybir.AluOpType.is_equal)
        msk = sb.tile([P, LE], fp32)
        nc.vector.select(msk[:], oh[:], eit[:], nines[:])
        nc.vector.tensor_reduce(
            out=eg_stk[:].rearrange("p (ti two) -> p ti two", two=2)[:, :, 0:1],
            in_=msk[:].rearrange("p (ti e) -> p ti e", e=E),
            axis=mybir.AxisListType.X, op=mybir.AluOpType.min)
        eidx_v = eg_stk[:].rearrange("p (ti two) -> p ti two", two=2)[:, :, 0:1]
        gate_v = eg_stk[:].rearrange("p (ti two) -> p ti two", two=2)[:, :, 1:2]
        mine_all = sb.tile([P, LE], fp32)
        nc.vector.tensor_tensor(
            out=mine_all[:].rearrange("p (ti e) -> p ti e", e=E),
            in0=eidx_v.to_broadcast([P, LT, E]),
            in1=eit[:].rearrange("p (ti e) -> p ti e", e=E),
            op=mybir.AluOpType.is_equal)

        # ---- local queue positions + per-expert counts (all shard-local)
        offsb = sb.tile([1, LE + E], fp32)
        palls = sb.tile([P, LE], fp32)
        with tc.tile_pool(name="ppb", bufs=1, space="PSUM") as ppb:
            pts = ppb.tile([LE, 1], fp32, tag="pts")
            nc.tensor.matmul(pts[:], lhsT=mine_all[:], rhs=onescol[:],
                             start=True, stop=True)
            tscol = sb.tile([LE, 1], fp32)
            nc.vector.tensor_copy(tscol[:], pts[:])
            poffs = ppb.tile([1, LE + E], fp32, tag="poffs")
            nc.tensor.matmul(poffs[:], lhsT=tscol[:], rhs=w64t[:],
                             start=True, stop=True)
            nc.vector.tensor_copy(offsb[:], poffs[:])
            # counts c_{me,e} -> row 0 of each region of the send buffer
            nc.sync.dma_start(
                igd_ps[0][:].rearrange("(e cl) two -> cl e two", cl=CL)
                [0:1, :, 0:1],
                offsb[:, LE:LE + E].rearrange("p (e one) -> p e one", one=1))
            pall = ppb.tile([P, LE], fp32, tag="pall")
            nc.tensor.matmul(pall[:], lhsT=trit[:], rhs=mine_all[:],
                             start=True, stop=False)
            nc.tensor.matmul(pall[:], lhsT=ones1[:], rhs=offsb[:, 0:LE],
                             start=False, stop=True)
            nc.vector.tensor_copy(palls[:], pall[:])
        mu8 = sb.tile([P, LE], dt.uint8)
        nc.vector.tensor_scalar(
            out=mu8[:], in0=mine_all[:], scalar1=0.5, scalar2=None,
            op0=mybir.AluOpType.is_gt)
        cu8 = sb.tile([P, LE], dt.uint8)
        nc.vector.tensor_scalar(
            out=cu8[:], in0=palls[:], scalar1=float(CL) - 0.5, scalar2=None,
            op0=mybir.AluOpType.is_lt)
        au8 = sb.tile([P, LE], dt.uint8)
        nc.vector.tensor_tensor(
            out=au8[:], in0=mu8[:], in1=cu8[:], op=mybir.AluOpType.mult)
        s1 = sb.tile([P, LE], fp32)
        nc.vector.select(s1[:], au8[:], palls[:], huget[:])
        dstf = sb.tile([P, LE], fp32)
        nc.vector.tensor_tensor(
            out=dstf[:], in0=s1[:], in1=eclt[:], op=mybir.AluOpType.add)
        rowmin = sb.tile([P, LT], fp32)
        nc.vector.tensor_reduce(
            out=rowmin[:].rearrange("p (ti one) -> p ti one", one=1),
            in_=dstf[:].rearrange("p (ti e) -> p ti e", e=E),
            axis=mybir.AxisListType.X, op=mybir.AluOpType.min)
        dsti = sb.tile([P, LT], dt.int32)
        nc.vector.tensor_copy(dsti[:], rowmin[:])
        pairs = sb.tile([P, LT * 2], fp32)
        nc.vector.tensor_copy(
            pairs[:].rearrange("p (t two) -> p t two", two=2)[:, :, 0:1],
            tokp1[:].rearrange("p (t one) -> p t one", one=1))
        nc.vector.tensor_copy(
            pairs[:].rearrange("p (t two) -> p t two", two=2)[:, :, 1:2],
            gate_v)

        # ---- scatter (id+1, gate) into per-(shard,expert) regions.
        # Four destination tensors -> four independent 2-link WAW chains
        # instead of one 8-link chain; merged below on the scalar queue
        # (kept off the sync queue, whose w1-stream buffer waits depend
        # transitively on this merge).
        for t in range(LT):
            nc.gpsimd.indirect_dma_start(
                out=igd_ps[t % 4][:], out_offset=bass.IndirectOffsetOnAxis(
                    ap=dsti[:, t:t + 1], axis=0),
                in_=pairs[:, 2 * t:2 * t + 2], in_offset=None,
                bounds_check=RCV - 1, oob_is_err=False)
        mrg = sb.tile([P, RCV * 2 // P], fp32)
        mrgb = sb.tile([P, RCV * 2 // P], fp32)
        mrgc = sb.tile([P, RCV * 2 // P], fp32)
        mrgd = sb.tile([P, RCV * 2 // P], fp32)
        for i, dst in enumerate((mrg, mrgb, mrgc, mrgd)):
            nc.scalar.dma_start(
                dst[:], igd_ps[i][:].rearrange("(p c) two -> p c two", p=P))
        nc.vector.tensor_tensor(
            out=mrg[:], in0=mrg[:], in1=mrgb[:], op=mybir.AluOpType.add)
        nc.vector.tensor_tensor(
            out=mrgc[:], in0=mrgc[:], in1=mrgd[:], op=mybir.AluOpType.add)
        nc.vector.tensor_tensor(
            out=mrg[:], in0=mrg[:], in1=mrgc[:], op=mybir.AluOpType.add)
        nc.scalar.dma_start(
            igd_loc[:].rearrange("(p c) two -> p c two", p=P), mrg[:])

        # ---- the single AllToAll: region e -> core e
        nc.gpsimd.collective_compute(
            "AllToAll", mybir.AluOpType.bypass,
            ins=[igd_loc[:]], outs=[igd_rcv[:]],
            replica_groups=[list(range(E))])

        # ---- A set: my own tokens for my expert (local pos < NAS).
        # Read them straight out of my own send buffer region.
        pairsA = sb.tile([P, NA * 2], fp32)
        nc.vector.memset(pairsA[:], 0.0)
        for c in range(NA):
            nc.gpsimd.indirect_dma_start(
                out=pairsA[:, c * 2:(c + 1) * 2], out_offset=None,
                in_=igd_loc[:], in_offset=bass.IndirectOffsetOnAxis(
                    ap=meloct[:, c:c + 1], axis=0),
                bounds_check=RCV - 1, oob_is_err=False)
        ivA = pairsA[:].rearrange("p (c two) -> p c two", two=2)[:, :, 0:1]
        gvA = pairsA[:].rearrange("p (c two) -> p c two", two=2)[:, :, 1:2]
        vA8 = sb.tile([P, NA], dt.uint8)
        nc.vector.tensor_scalar(
            out=vA8[:], in0=ivA, scalar1=0.5, scalar2=None,
            op0=mybir.AluOpType.is_gt)
        idm1A = sb.tile([P, NA], fp32)
        nc.vector.tensor_scalar_add(
            idm1A[:].rearrange("p (c one) -> p c one", one=1), ivA, -1.0)
        idxfA = sb.tile([P, NA], fp32)
        nc.vector.select(idxfA[:], vA8[:], idm1A[:], bigA[:])
        idxAin = sb.tile([P, NA], dt.int32)
        nc.vector.tensor_copy(idxAin[:], idxfA[:])
        gateA = sb.tile([P, NA], fp32)
        nc.vector.tensor_copy(
            gateA[:].rearrange("p (c one) -> p c one", one=1), gvA)

        w2t = sb.tile([P, DC * M], bf16)
        hT_B = sb.tile([P, DC * NBS], bf16)
        # A's hidden activations alias the first DC*NAS columns of hT_B:
        # A-w2's reads complete exactly when B-w1's writes begin.
        hT_A = hT_B
        dispT_A = sb.tile([P, MC * NAS], bf16)
        dispT_B = sb.tile([P, MC * NBS], bf16)

        with (
            tc.tile_pool(name="psT", bufs=2, space="PSUM") as psT,
            tc.tile_pool(name="psW", bufs=2, space="PSUM") as psW,
            tc.tile_pool(name="ps2", bufs=2, space="PSUM") as ps2,
        ):
            # ---- gather A tokens + transpose into dispT_A
            for c in range(NA):
                gx = sbg.tile([P, M], bf16, tag="gx")
                nc.vector.memset(gx[:], 0.0)
                nc.gpsimd.indirect_dma_start(
                    out=gx[:], out_offset=None, in_=xb[:],
                    in_offset=bass.IndirectOffsetOnAxis(
                        ap=idxAin[:, c:c + 1], axis=0),
                    bounds_check=T - 1, oob_is_err=False)
                for mm in range(MC):
                    ptg = psT.tile([P, P], fp32, tag="ptg")
                    nc.tensor.matmul(
                        ptg[:], lhsT=gx[:, mm * P:(mm + 1) * P],
                        rhs=idb[:], start=True, stop=True)
                    nc.vector.tensor_copy(
                        dispT_A[:, mm * NAS + c * P:mm * NAS + (c + 1) * P],
                        ptg[:])
            # ---- A first layer (w1 stream pass 1)
            for d in range(DC):
                w1t = sbw1.tile([P, M], bf16, tag="w1t")
                nc.sync.dma_start(w1t[:], w1p[d])
                pA = psW.tile([P, NAS], fp32, tag="pA")
                for mc in range(MC):
                    nc.tensor.matmul(
                        pA[:], lhsT=w1t[:, mc * P:(mc + 1) * P],
                        rhs=dispT_A[:, mc * NAS:(mc + 1) * NAS],
                        start=(mc == 0), stop=(mc == MC - 1))
                nc.scalar.activation(
                    hT_A[:, d * NAS:(d + 1) * NAS], pA[:],
                    mybir.ActivationFunctionType.Relu,
                    bias=b1t[:, d:d + 1], scale=1.0)

            # ---- w2 resident load (sync queue: after the A w1 stream)
            for q in range(4):
                nc.sync.dma_start(
                    w2t[:, q * 8 * M:(q + 1) * 8 * M],
                    w2p[:, q * 8:(q + 1) * 8, :])

            # ---- B-prep: counts -> prefix sums -> per-slot src index + pos
            cnt8 = sb.tile([1, E], fp32)
            nc.gpsimd.dma_start(
                cnt8[:].rearrange("p (a s) -> p a s", a=1),
                igd_rcv[:].rearrange("(s cl) two -> two cl s", cl=CL)
                [0:1, 0:1, :])
            mem8 = percs[:, 0:8]
            bvec = percs[:, 8:16]
            r0m1 = percs[:, 16:24]
            cprime = sb.tile([1, E], fp32)
            nc.vector.tensor_scalar(
                out=cprime[:], in0=mem8, scalar1=float(-NAS), scalar2=None,
                op0=mybir.AluOpType.mult)
            nc.vector.tensor_tensor(
                out=cprime[:], in0=cnt8[:], in1=cprime[:],
                op=mybir.AluOpType.add)
            nc.vector.tensor_scalar(
                out=cprime[:], in0=cprime[:], scalar1=0.0, scalar2=None,
                op0=mybir.AluOpType.max)

            def _incl_prefix(src):
                a1 = sb.tile([1, E], fp32)
                nc.vector.tensor_copy(a1[:], src[:])
                nc.vector.tensor_tensor(
                    out=a1[:, 1:8], in0=src[:, 1:8], in1=src[:, 0:7],
                    op=mybir.AluOpType.add)
                a2 = sb.tile([1, E], fp32)
                nc.vector.tensor_copy(a2[:], a1[:])
                nc.vector.tensor_tensor(
                    out=a2[:, 2:8], in0=a1[:, 2:8], in1=a1[:, 0:6],
                    op=mybir.AluOpType.add)
                a3 = sb.tile([1, E], fp32)
                nc.vector.tensor_copy(a3[:], a2[:])
                nc.vector.tensor_tensor(
                    out=a3[:, 4:8], in0=a2[:, 4:8], in1=a2[:, 0:4],
                    op=mybir.AluOpType.add)
                ex_ = sb.tile([1, E], fp32)
                nc.vector.memset(ex_[:], 0.0)
                nc.vector.tensor_copy(ex_[:, 1:8], a3[:, 0:7])
                return ex_

            offx = _incl_prefix(cnt8)     # exclusive prefix of full counts
            boff = _incl_prefix(cprime)   # exclusive prefix of B counts
            boffm = sb.tile([1, E], fp32)
            nc.vector.tensor_scalar_add(boffm[:], boff[:], -0.5)
            srcv = sb.tile([1, E], fp32)
            nc.vector.tensor_tensor(
                out=srcv[:], in0=bvec, in1=boff[:],
                op=mybir.AluOpType.subtract)
            posoffv = sb.tile([1, E], fp32)
            nc.vector.tensor_tensor(
                out=posoffv[:], in0=offx[:], in1=r0m1,
                op=mybir.AluOpType.add)
            nc.vector.tensor_tensor(
                out=posoffv[:], in0=posoffv[:], in1=boff[:],
                op=mybir.AluOpType.subtract)
            offme1 = sb.tile([1, E], fp32)
            nc.vector.tensor_tensor(
                out=offme1[:], in0=mem8, in1=offx[:],
                op=mybir.AluOpType.mult)
            scrt = sb.tile([1, 32], fp32)
            nc.vector.tensor_copy(scrt[:, 0:8], boffm[:])
            nc.vector.tensor_copy(scrt[:, 8:16], srcv[:])
            nc.vector.tensor_copy(scrt[:, 16:24], posoffv[:])
            nc.vector.tensor_reduce(
                out=scrt[:, 24:25].rearrange("p (a s) -> p a s", a=1),
                in_=offme1[:].rearrange("p (a s) -> p a s", a=1),
                axis=mybir.AxisListType.X, op=mybir.AluOpType.add)
            nc.gpsimd.dma_start(scrd[:], scrt[:])
            bct = sb.tile([P, 32], fp32)
            nc.gpsimd.dma_start(bct[:], scrd[:].to_broadcast([P, 32]))

            q3 = qiot[:].rearrange("p (c one) -> p c one", one=1) \
                .to_broadcast([P, NB, E])
            bof3 = bct[:, 0:8].rearrange("p (one s) -> p one s", one=1) \
                .to_broadcast([P, NB, E])
            src3 = bct[:, 8:16].rearrange("p (one s) -> p one s", one=1) \
                .to_broadcast([P, NB, E])
            pos3 = bct[:, 16:24].rearrange("p (one s) -> p one s", one=1) \
                .to_broadcast([P, NB, E])
            m3 = sb.tile([P, NB * E], fp32)
            nc.vector.tensor_tensor(
                out=m3[:].rearrange("p (c s) -> p c s", s=E), in0=q3, in1=bof3,
                op=mybir.AluOpType.is_gt)
            t3 = sb.tile([P, NB * E], fp32)
            nc.vector.tensor_tensor(
                out=t3[:].rearrange("p (c s) -> p c s", s=E),
                in0=m3[:].rearrange("p (c s) -> p c s", s=E), in1=src3,
                op=mybir.AluOpType.mult)
            srcq = sb.tile([P, NB], fp32)
            nc.vector.tensor_reduce(
                out=srcq[:].rearrange("p (c one) -> p c one", one=1),
                in_=t3[:].rearrange("p (c s) -> p c s", s=E),
                axis=mybir.AxisListType.X, op=mybir.AluOpType.max)
            nc.vector.tensor_tensor(
                out=srcq[:], in0=srcq[:], in1=qiot[:],
                op=mybir.AluOpType.add)
            srci = sb.tile([P, NB], dt.int32)
            nc.vector.tensor_copy(srci[:], srcq[:])
            # exact region id from src: s = floor(src / CL)
            sqf = sb.tile([P, NB], fp32)
            nc.vector.tensor_scalar(
                out=sqf[:], in0=srcq[:], scalar1=1.0 / CL, scalar2=None,
                op0=mybir.AluOpType.mult)
            sqi = sb.tile([P, NB], dt.int32)
            nc.vector.tensor_copy(sqi[:], sqf[:])
            nc.vector.tensor_copy(sqf[:], sqi[:])
            m2 = sb.tile([P, NB * E], fp32)
            nc.vector.tensor_tensor(
                out=m2[:].rearrange("p (c s) -> p c s", s=E),
                in0=sqf[:].rearrange("p (c one) -> p c one", one=1)
                .to_broadcast([P, NB, E]),
                in1=siot[:].rearrange("p (one s) -> p one s", one=1)
                .to_broadcast([P, NB, E]),
                op=mybir.AluOpType.is_equal)
            nc.vector.tensor_tensor(
                out=m2[:].rearrange("p (c s) -> p c s", s=E),
                in0=m2[:].rearrange("p (c s) -> p c s", s=E), in1=pos3,
                op=mybir.AluOpType.mult)
            posq = sb.tile([P, NB], fp32)
            nc.vector.tensor_reduce(
                out=posq[:].rearrange("p (c one) -> p c one", one=1),
                in_=m2[:].rearrange("p (c s) -> p c s", s=E),
                axis=mybir.AxisListType.X, op=mybir.AluOpType.max)
            nc.vector.tensor_tensor(
                out=posq[:], in0=posq[:], in1=qiot[:],
                op=mybir.AluOpType.add)

            # ---- compaction gather of (id+1, gate) pairs for the B set
            pairsB = sb.tile([P, NB * 2], fp32)
            nc.vector.memset(pairsB[:], 0.0)
            for c in range(NB):
                nc.gpsimd.indirect_dma_start(
                    out=pairsB[:, c * 2:(c + 1) * 2], out_offset=None,
                    in_=igd_rcv[:], in_offset=bass.IndirectOffsetOnAxis(
                        ap=srci[:, c:c + 1], axis=0),
                    bounds_check=RCV - 1, oob_is_err=False)
            ivB = pairsB[:].rearrange("p (c two) -> p c two", two=2)[:, :, 0:1]
            gvB = pairsB[:].rearrange("p (c two) -> p c two", two=2)[:, :, 1:2]
            vB8 = sb.tile([P, NB], dt.uint8)
            nc.vector.tensor_scalar(
                out=vB8[:], in0=ivB, scalar1=0.5, scalar2=None,
                op0=mybir.AluOpType.is_gt)
            keep8 = sb.tile([P, NB], dt.uint8)
            nc.vector.tensor_scalar(
                out=keep8[:], in0=posq[:], scalar1=float(C) - 0.5, scalar2=None,
                op0=mybir.AluOpType.is_lt)
            nc.vector.tensor_tensor(
                out=vB8[:], in0=vB8[:], in1=keep8[:],
                op=mybir.AluOpType.mult)
            idm1B = sb.tile([P, NB], fp32)
            nc.vector.tensor_scalar_add(
                idm1B[:].rearrange("p (c one) -> p c one", one=1), ivB, -1.0)
            idxfB = sb.tile([P, NB], fp32)
            nc.vector.select(idxfB[:], vB8[:], idm1B[:], bigB[:])
            idxB = sb.tile([P, NB], dt.int32)
            nc.vector.tensor_copy(idxB[:], idxfB[:])
            gateB = sb.tile([P, NB], fp32)
            nc.vector.tensor_copy(
                gateB[:].rearrange("p (c one) -> p c one", one=1), gvB)

            # ---- gather B tokens + transpose into dispT_B
            for c in range(NB):
                gx = sbg.tile([P, M], bf16, tag="gx")
                nc.vector.memset(gx[:], 0.0)
                nc.gpsimd.indirect_dma_start(
                    out=gx[:], out_offset=None, in_=xb[:],
                    in_offset=bass.IndirectOffsetOnAxis(
                        ap=idxB[:, c:c + 1], axis=0),
                    bounds_check=T - 1, oob_is_err=False)
                for mm in range(MC):
                    ptg = psT.tile([P, P], fp32, tag="ptg")
                    nc.tensor.matmul(
                        ptg[:], lhsT=gx[:, mm * P:(mm + 1) * P],
                        rhs=idb[:], start=True, stop=True)
                    nc.vector.tensor_copy(
                        dispT_B[:, mm * NBS + c * P:mm * NBS + (c + 1) * P],
                        ptg[:])

            # ---- A-drop mask (uses off_me, available post-A2A) + A second
            # layer + output scatter
            posA = sb.tile([P, NA], fp32)
            nc.vector.tensor_tensor(
                out=posA[:].rearrange("p (c one) -> p c one", one=1),
                in0=qiot[:, 0:NA].rearrange("p (c one) -> p c one", one=1),
                in1=bct[:, 24:25].rearrange("p (c one) -> p c one", one=1)
                .to_broadcast([P, NA, 1]),
                op=mybir.AluOpType.add)
            keepA = sb.tile([P, NA], dt.uint8)
            nc.vector.tensor_scalar(
                out=keepA[:], in0=posA[:], scalar1=float(C) - 0.5, scalar2=None,
                op0=mybir.AluOpType.is_lt)
            nc.vector.tensor_tensor(
                out=keepA[:], in0=keepA[:], in1=vA8[:],
                op=mybir.AluOpType.mult)
            idxfAo = sb.tile([P, NA], fp32)
            nc.vector.select(idxfAo[:], keepA[:], idm1A[:], bigA[:])
            idxAo = sb.tile([P, NA], dt.int32)
            nc.vector.tensor_copy(idxAo[:], idxfAo[:])

            for s5 in range(NA):
                po0 = ps2.tile([P, 512], fp32, tag="po")
                po1 = ps2.tile([P, 512], fp32, tag="po")
                for d in range(DC):
                    lhs = hT_A[:, d * NAS + s5 * P:d * NAS + (s5 + 1) * P]
                    nc.tensor.matmul(
                        po0[:], lhsT=lhs, rhs=w2t[:, d * M:d * M + 512],
                        start=(d == 0), stop=(d == DC - 1))
                    nc.tensor.matmul(
                        po1[:], lhsT=lhs, rhs=w2t[:, d * M + 512:(d + 1) * M],
                        start=(d == 0), stop=(d == DC - 1))
                st = sbst.tile([P, M], fp32, tag="st")
                for hh, po in ((0, po0), (1, po1)):
                    nc.vector.tensor_tensor(
                        out=st[:, hh * 512:(hh + 1) * 512], in0=po[:],
                        in1=b2t[:, hh * 512:(hh + 1) * 512],
                        op=mybir.AluOpType.add)
                nc.vector.tensor_scalar_mul(
                    st[:], st[:], gateA[:, s5:s5 + 1])
                nc.gpsimd.indirect_dma_start(
                    out=outd[:], out_offset=bass.IndirectOffsetOnAxis(
                        ap=idxAo[:, s5:s5 + 1], axis=0),
                    in_=st[:], in_offset=None,
                    bounds_check=T - 1, oob_is_err=False)

            # ---- B first layer (w1 stream pass 2)
            for d in range(DC):
                w1t = sbw1.tile([P, M], bf16, tag="w1t")
                nc.sync.dma_start(w1t[:], w1p[d])
                pA = psW.tile([P, 512], fp32, tag="pA")
                pB = psW.tile([P, 512], fp32, tag="pB")
                for mc in range(MC):
                    lhs = w1t[:, mc * P:(mc + 1) * P]
                    nc.tensor.matmul(
                        pA[:], lhsT=lhs,
                        rhs=dispT_B[:, mc * NBS:mc * NBS + 512],
                        start=(mc == 0), stop=(mc == MC - 1))
                    nc.tensor.matmul(
                        pB[:], lhsT=lhs,
                        rhs=dispT_B[:, mc * NBS + 512:(mc + 1) * NBS],
                        start=(mc == 0), stop=(mc == MC - 1))
                nc.scalar.activation(
                    hT_B[:, d * NBS:d * NBS + 512], pA[:],
                    mybir.ActivationFunctionType.Relu,
                    bias=b1t[:, d:d + 1], scale=1.0)
                nc.scalar.activation(
                    hT_B[:, d * NBS + 512:(d + 1) * NBS], pB[:],
                    mybir.ActivationFunctionType.Relu,
                    bias=b1t[:, d:d + 1], scale=1.0)

            # ---- B second layer + output scatter
            for s5 in range(NB):
                po0 = ps2.tile([P, 512], fp32, tag="po")
                po1 = ps2.tile([P, 512], fp32, tag="po")
                for d in range(DC):
                    lhs = hT_B[:, d * NBS + s5 * P:d * NBS + (s5 + 1) * P]
                    nc.tensor.matmul(
                        po0[:], lhsT=lhs, rhs=w2t[:, d * M:d * M + 512],
                        start=(d == 0), stop=(d == DC - 1))
                    nc.tensor.matmul(
                        po1[:], lhsT=lhs, rhs=w2t[:, d * M + 512:(d + 1) * M],
                        start=(d == 0), stop=(d == DC - 1))
                st = sbst.tile([P, M], fp32, tag="st")
                for hh, po in ((0, po0), (1, po1)):
                    nc.vector.tensor_tensor(
                        out=st[:, hh * 512:(hh + 1) * 512], in0=po[:],
                        in1=b2t[:, hh * 512:(hh + 1) * 512],
                        op=mybir.AluOpType.add)
                nc.vector.tensor_scalar_mul(
                    st[:], st[:], gateB[:, s5:s5 + 1])
                nc.gpsimd.indirect_dma_start(
                    out=outd[:], out_offset=bass.IndirectOffsetOnAxis(
                        ap=idxB[:, s5:s5 + 1], axis=0),
                    in_=st[:], in_offset=None,
                    bounds_check=T - 1, oob_is_err=False)

    nc.compile()
    return nc


def _make_w64():
    w = np.zeros((LE, LE + E), dtype=np.float32)
    for tip in range(LT):
        for ep in range(E):
            r = tip * E + ep
            for ti in range(LT):
                if tip < ti:
                    w[r, ti * E + ep] = 1.0
            w[r, LE + ep] = 1.0
    return w


def _prep_inputs(x, wg, w1, b1, w2, b2):
    bf = ml_dtypes.bfloat16
    tokens = np.ascontiguousarray(x.reshape(T, M)).astype(np.float32)
    xT = np.ascontiguousarray(tokens.T)
    xb = tokens.astype(bf)
    wgf = np.ascontiguousarray(wg.astype(np.float32))
    eiota = np.tile(np.arange(E, dtype=np.float32), LT)[None, :].repeat(P, 0)
    triu = np.triu(np.ones((P, P), dtype=np.float32))
    identf = np.eye(P, dtype=np.float32)
    identb = np.eye(P).astype(bf)
    w64 = _make_w64()
    ecl = np.tile(np.arange(E, dtype=np.float32) * CL, LT)[None, :].repeat(P, 0)
    qiota = (np.arange(NB, dtype=np.float32)[None, :] * P
             + np.arange(P, dtype=np.float32)[:, None]).copy()
    siota = np.arange(E, dtype=np.float32)[None, :].repeat(P, 0)
    cpak = np.concatenate(
        [eiota, triu, identf, ecl,
         np.zeros((P, 8), np.float32), qiota, siota], axis=1)
    in_maps = []
    for e in range(E):
        w1e = np.ascontiguousarray(w1[e]).astype(bf)
        w1pk = np.ascontiguousarray(
            w1e.reshape(MC, P, DC, P).transpose(2, 1, 0, 3))
        w2e = np.ascontiguousarray(w2[e]).astype(bf)
        w2pk = np.ascontiguousarray(
            w2e.reshape(DC, P, M).transpose(1, 0, 2))
        tokp1 = (e * TSH + np.arange(TSH, dtype=np.float32)
                 .reshape(LT, P).T + 1.0).copy()
        cpk = cpak.copy()
        cpk[:, 384:392] = tokp1
        meloc = (e * CL + 1 + qiota[:, :NA]).astype(np.int32)
        perc = np.zeros((1, 24), dtype=np.float32)
        for s in range(E):
            perc[0, s] = 1.0 if s == e else 0.0
            perc[0, 8 + s] = s * CL + (NAS + 1 if s == e else 1)
            perc[0, 16 + s] = float(NAS) if s == e else 0.0
        in_maps.append({
            "xTs": np.ascontiguousarray(xT[:, e * TSH:(e + 1) * TSH]),
            "xb": xb, "wg": wgf,
            "w1p": w1pk, "w2p": w2pk,
            "b1v": np.ascontiguousarray(b1[e]).astype(np.float32),
            "b2b": np.tile(np.asarray(b2[e], dtype=np.float32), (P, 1)),
            "cpakd": np.ascontiguousarray(cpk),
            "identb": identb, "w64d": w64,
            "melocd": meloc, "percd": perc,
        })
    return in_maps


def kernel(x, wg, w1, b1, w2, b2, _trace=False):
    if "nc" not in _CACHE:
        _CACHE["nc"] = _build_nc()
    nc = _CACHE["nc"]
    in_maps = _prep_inputs(
        np.asarray(x), np.asarray(wg), np.asarray(w1),
        np.asarray(b1), np.asarray(w2), np.asarray(b2))
    res = run_bass_kernel_spmd(nc, in_maps, list(range(E)), trace=_trace)
    _CACHE["last_results"] = res
    full = np.zeros((T, M), dtype=np.float32)
    for e in range(E):
        full += res.results[e]["out"]
    return full.reshape(B, S, M)


# revision 31
# speedup vs baseline: 1.0042x; 1.0042x over previous
"""MoE top-1 routing kernel for 8 TRN2 NeuronCores (expert parallelism).

Self-contained: takes full inputs, shards experts across 8 cores, returns the
full output (host sums the 8 disjoint per-expert partials).

v5 design (local-first pipelining, single collective):
- Gating is token-sharded: each core computes fp32 logits for its own 1024
  tokens, then DVE softmax/argmax and local (shard-internal) queue positions
  via one triangular matmul. No counts-AllGather is needed: each shard
  scatters (token_id+1, gate) into per-(shard,expert) regions of a [E*CL,2]
  send buffer at LOCAL positions (CL=512 rows/region, row 0 = count), and one
  32 KB AllToAll delivers region e to core e.
- Latency hiding: tokens of a core's OWN shard choosing its OWN expert (the
  "A set", local pos < 256) are known before the AllToAll; the core gathers
  them from the replicated token buffer and starts the FFN on them (2 chunks
  of 128 slots) while the collective + receive-side compaction for the
  remaining "B set" (8 chunks) is still in flight. Queue order is irrelevant
  to the math: outputs are scattered back by token id, and capacity drops
  (global pos >= C) are applied exactly at output-scatter time using the
  received per-shard counts.
- Receive-side compaction is a computed gather: per-shard counts (embedded at
  region row 0) -> prefix sums on DVE -> per-slot source index + global
  position -> one level of 8B-row indirect gathers, then the usual 2KB-row
  token gathers from the bf16 token buffer.
- FFN in bf16: w2 resident in SBUF (8 MB), w1 streamed once per pass (A then
  B), fused bias+ReLU on the scalar engine, gate-scaled rows scattered into
  the output by token id.
"""
import numpy as np
import ml_dtypes
from contextlib import ExitStack

import concourse.bass as bass
import concourse.tile as tile
from concourse import bacc, mybir
from concourse.bass_utils import run_bass_kernel_spmd

dt = mybir.dt

B, S, M, E, DFF = 4, 2048, 1024, 8, 4096
T = B * S
C = int(1.25 * T / E)          # 1280 capacity per expert
P = 128
MC = M // P                    # 8
DC = DFF // P                  # 32
TSH = T // E                   # 1024 tokens per shard
LT = TSH // P                  # 8
LE = LT * E                    # 64
CL = 512                       # rows per (shard, expert) region (row0=count)
RCV = E * CL                   # 4096
NA = 2                         # local-first chunks (256 slots)
NAS = NA * P
NB = 8                         # remote/compacted chunks (1024 slots)
NBS = NB * P
BIG = 1.5e9

_CACHE = {}


def _build_nc():
    nc = bacc.Bacc("TRN2", target_bir_lowering=False, debug=False)

    xTs = nc.dram_tensor("xTs", [M, TSH], dt.float32, kind="ExternalInput")
    xb = nc.dram_tensor("xb", [T, M], dt.bfloat16, kind="ExternalInput")
    wg = nc.dram_tensor("wg", [M, E], dt.float32, kind="ExternalInput")
    w1p = nc.dram_tensor("w1p", [DC, P, MC, P], dt.bfloat16, kind="ExternalInput")
    w2p = nc.dram_tensor("w2p", [P, DC, M], dt.bfloat16, kind="ExternalInput")
    b1v = nc.dram_tensor("b1v", [DFF], dt.float32, kind="ExternalInput")
    b2b = nc.dram_tensor("b2b", [P, M], dt.float32, kind="ExternalInput")
    cpakd = nc.dram_tensor("cpakd", [P, 408], dt.float32, kind="ExternalInput")
    identb = nc.dram_tensor("identb", [P, P], dt.bfloat16, kind="ExternalInput")
    w64d = nc.dram_tensor("w64d", [LE, LE + E], dt.float32, kind="ExternalInput")
    melocd = nc.dram_tensor("melocd", [P, NA], dt.int32, kind="ExternalInput")
    percd = nc.dram_tensor("percd", [1, 24], dt.float32, kind="ExternalInput")
    outd = nc.dram_tensor("out", [T, M], dt.float32, kind="ExternalOutput")

    igd_loc = nc.dram_tensor("igd_loc", [RCV, 2], dt.float32)
    igd_ps = [nc.dram_tensor(f"igd_p{i}", [RCV, 2], dt.float32)
              for i in range(4)]
    igd_rcv = nc.dram_tensor("igd_rcv", [RCV, 2], dt.float32)
    scrd = nc.dram_tensor("scrd", [1, 32], dt.float32)
    wrm_l = nc.dram_tensor("wrm_l", [8, 2], dt.float32)
    wrm_a = nc.dram_tensor("wrm_a", [64, 2], dt.float32, addr_space="Shared")

    fp32 = dt.float32
    bf16 = dt.bfloat16

    with tile.TileContext(nc) as tc, ExitStack() as ctx:
        sb = ctx.enter_context(tc.tile_pool(name="sb", bufs=1))
        sbx = ctx.enter_context(tc.tile_pool(name="sbx", bufs=6))
        sbw1 = ctx.enter_context(tc.tile_pool(name="sbw1", bufs=4))
        sbg = ctx.enter_context(tc.tile_pool(name="sbg", bufs=4))
        sbst = ctx.enter_context(tc.tile_pool(name="sbst", bufs=2))

        # ---- warmup collective: absorbs CC channel init + start barrier
        wz = sb.tile([8, 2], fp32)
        nc.vector.memset(wz[:], 0.0)
        nc.sync.dma_start(wrm_l[:], wz[:])
        nc.gpsimd.collective_compute(
            "AllGather", mybir.AluOpType.bypass,
            ins=[wrm_l[:]], outs=[wrm_a[:]],
            replica_groups=[list(range(E))])

        # ---- const loads (sync queue; xTs chunks follow in the gating loop)
        wgt = sb.tile([P, MC * E], fp32)
        nc.sync.dma_start(wgt[:], wg[:].rearrange("(mc p) e -> p mc e", p=P))
        cpak = sb.tile([P, 408], fp32)
        nc.sync.dma_start(cpak[:], cpakd[:])
        eit = cpak[:, 0:64]
        trit = cpak[:, 64:192]
        idf = cpak[:, 192:320]
        eclt = cpak[:, 320:384]
        tokp1 = cpak[:, 384:392]
        qiot = cpak[:, 392:400]
        siot = cpak[:, 400:408]
        idb = sb.tile([P, P], bf16)
        nc.sync.dma_start(idb[:], identb[:])
        w64t = sb.tile([LE, LE + E], fp32)
        nc.sync.dma_start(w64t[:], w64d[:])
        meloct = sb.tile([P, NA], dt.int32)
        nc.sync.dma_start(meloct[:], melocd[:])
        percs = sb.tile([1, 24], fp32)
        nc.sync.dma_start(percs[:], percd[:])
        b1t = sb.tile([P, DC], fp32)
        nc.sync.dma_start(b1t[:], b1v[:].rearrange("(d p) -> p d", p=P))
        # PE warm-up: keep the HAM activity window busy before the logits MMs
        wrmj = sb.tile([P, P], bf16)
        nc.vector.memset(wrmj[:], 0.0)
        # scalar queue: b2 broadcast + zero-prefill of the send buffer
        b2t = sb.tile([P, M], fp32)
        nc.scalar.dma_start(b2t[:], b2b[:])
        zpre = sb.tile([P, RCV * 2 // P], fp32)
        nc.vector.memset(zpre[:], 0.0)
        for i in range(4):
            nc.scalar.dma_start(
                igd_ps[i][:].rearrange("(p c) two -> p c two", p=P), zpre[:])

        ones1 = sb.tile([1, P], fp32)
        nc.vector.memset(ones1[:], 1.0)
        onescol = sb.tile([P, 1], fp32)
        nc.vector.memset(onescol[:], 1.0)
        nines = sb.tile([P, LE], fp32)
        nc.vector.memset(nines[:], 9.0)
        huget = sb.tile([P, LE], fp32)
        nc.vector.memset(huget[:], BIG)
        bigA = sb.tile([P, NA], fp32)
        nc.vector.memset(bigA[:], BIG)
        bigB = sb.tile([P, NB], fp32)
        nc.vector.memset(bigB[:], BIG)

        # ---- gating: fp32 logits for my 1024 tokens
        eg_stk = sb.tile([P, LT * 2], fp32)
        lg_stk = sb.tile([P, LE], fp32)
        lgT = sb.tile([8, TSH], fp32)
        with tc.tile_pool(name="psg", bufs=2, space="PSUM") as psg:
            pwrm = psg.tile([P, P], fp32, tag="pwrm")
            for _w in range(24):
                nc.tensor.matmul(pwrm[:], lhsT=wrmj[:], rhs=idb[:],
                                 start=(_w == 0), stop=(_w == 23))
            pl0 = psg.tile([8, 512], fp32, tag="pl0")
            pl1 = psg.tile([8, 512], fp32, tag="pl1")
            for k in range(MC):
                xt0 = sbx.tile([P, 512], fp32, tag="xt")
                nc.sync.dma_start(xt0[:], xTs[k * P:(k + 1) * P, 0:512])
                nc.tensor.matmul(
                    pl0[:], lhsT=wgt[:, k * E:(k + 1) * E], rhs=xt0[:],
                    start=(k == 0), stop=(k == MC - 1))
                xt1 = sbx.tile([P, 512], fp32, tag="xt")
                nc.sync.dma_start(xt1[:], xTs[k * P:(k + 1) * P, 512:1024])
                nc.tensor.matmul(
                    pl1[:], lhsT=wgt[:, k * E:(k + 1) * E], rhs=xt1[:],
                    start=(k == 0), stop=(k == MC - 1))
            nc.vector.tensor_copy(lgT[:, 0:512], pl0[:])
            nc.vector.tensor_copy(lgT[:, 512:1024], pl1[:])
            for ti in range(LT):
                pq = psg.tile([P, E], fp32, tag="pq")
                nc.tensor.transpose(
                    out=pq[:], in_=lgT[:, ti * P:(ti + 1) * P],
                    identity=idf[:8, :8])
                nc.vector.tensor_copy(lg_stk[:, ti * E:(ti + 1) * E], pq[:])
        lg3 = lg_stk[:].rearrange("p (ti e) -> p ti e", e=E)
        mx_stk = sb.tile([P, LT], fp32)
        nc.vector.tensor_reduce(
            out=mx_stk[:], in_=lg3, axis=mybir.AxisListType.X,
            op=mybir.AluOpType.max)
        mxb = mx_stk[:].rearrange("p (ti one) -> p ti one", one=1).to_broadcast([P, LT, E])
        ls = sb.tile([P, LE], fp32)
        nc.vector.tensor_tensor(
            out=ls[:].rearrange("p (ti e) -> p ti e", e=E), in0=lg3, in1=mxb,
            op=mybir.AluOpType.subtract)
        ex = sb.tile([P, LE], fp32)
        nc.scalar.activation(ex[:], ls[:], mybir.ActivationFunctionType.Exp)
        s_stk = sb.tile([P, LT], fp32)
        nc.vector.tensor_reduce(
            out=s_stk[:], in_=ex[:].rearrange("p (ti e) -> p ti e", e=E),
            axis=mybir.AxisListType.X, op=mybir.AluOpType.add)
        nc.vector.reciprocal(
            eg_stk[:].rearrange("p (ti two) -> p ti two", two=2)[:, :, 1:2],
            s_stk[:].rearrange("p (ti one) -> p ti one", one=1))
        oh = sb.tile([P, LE], dt.uint8)
        nc.vector.tensor_tensor(
            out=oh[:].rearrange("p (ti e) -> p ti e", e=E), in0=lg3, in1=mxb,
            op=mybir.AluOpType.is_equal)
        msk = sb.tile([P, LE], fp32)
        nc.vector.select(msk[:], oh[:], eit[:], nines[:])
        nc.vector.tensor_reduce(
            out=eg_stk[:].rearrange("p (ti two) -> p ti two", two=2)[:, :, 0:1],
            in_=msk[:].rearrange("p (ti e) -> p ti e", e=E),
            axis=mybir.AxisListType.X, op=mybir.AluOpType.min)
        eidx_v = eg_stk[:].rearrange("p (ti two) -> p ti two", two=2)[:, :, 0:1]
        gate_v = eg_stk[:].rearrange("p (ti two) -> p ti two", two=2)[:, :, 1:2]
        mine_all = sb.tile([P, LE], fp32)
        nc.vector.tensor_tensor(
            out=mine_all[:].rearrange("p (ti e) -> p ti e", e=E),
            in0=eidx_v.to_broadcast([P, LT, E]),
            in1=eit[:].rearrange("p (ti e) -> p ti e", e=E),
            op=mybir.AluOpType.is_equal)

        # ---- local queue positions + per-expert counts (all shard-local)
        offsb = sb.tile([1, LE + E], fp32)
        palls = sb.tile([P, LE], fp32)
        with tc.tile_pool(name="ppb", bufs=1, space="PSUM") as ppb:
            pts = ppb.tile([LE, 1], fp32, tag="pts")
            nc.tensor.matmul(pts[:], lhsT=mine_all[:], rhs=onescol[:],
                             start=True, stop=True)
            tscol = sb.tile([LE, 1], fp32)
            nc.vector.tensor_copy(tscol[:], pts[:])
            poffs = ppb.tile([1, LE + E], fp32, tag="poffs")
            nc.tensor.matmul(poffs[:], lhsT=tscol[:], rhs=w64t[:],
                             start=True, stop=True)
            nc.vector.tensor_copy(offsb[:], poffs[:])
            # counts c_{me,e} -> row 0 of each region of the send buffer
            nc.sync.dma_start(
                igd_ps[0][:].rearrange("(e cl) two -> cl e two", cl=CL)
                [0:1, :, 0:1],
                offsb[:, LE:LE + E].rearrange("p (e one) -> p e one", one=1))
            pall = ppb.tile([P, LE], fp32, tag="pall")
            nc.tensor.matmul(pall[:], lhsT=trit[:], rhs=mine_all[:],
                             start=True, stop=False)
            nc.tensor.matmul(pall[:], lhsT=ones1[:], rhs=offsb[:, 0:LE],
                             start=False, stop=True)
            nc.vector.tensor_copy(palls[:], pall[:])
        mu8 = sb.tile([P, LE], dt.uint8)
        nc.vector.tensor_scalar(
            out=mu8[:], in0=mine_all[:], scalar1=0.5, scalar2=None,
            op0=mybir.AluOpType.is_gt)
        cu8 = sb.tile([P, LE], dt.uint8)
        nc.vector.tensor_scalar(
            out=cu8[:], in0=palls[:], scalar1=float(CL) - 0.5, scalar2=None,
            op0=mybir.AluOpType.is_lt)
        au8 = sb.tile([P, LE], dt.uint8)
        nc.vector.tensor_tensor(
            out=au8[:], in0=mu8[:], in1=cu8[:], op=mybir.AluOpType.mult)
        s1 = sb.tile([P, LE], fp32)
        nc.vector.select(s1[:], au8[:], palls[:], huget[:])
        dstf = sb.tile([P, LE], fp32)
        nc.vector.tensor_tensor(
            out=dstf[:], in0=s1[:], in1=eclt[:], op=mybir.AluOpType.add)
        rowmin = sb.tile([P, LT], fp32)
        nc.vector.tensor_reduce(
            out=rowmin[:].rearrange("p (ti one) -> p ti one", one=1),
            in_=dstf[:].rearrange("p (ti e) -> p ti e", e=E),
            axis=mybir.AxisListType.X, op=mybir.AluOpType.min)
        dsti = sb.tile([P, LT], dt.int32)
        nc.vector.tensor_copy(dsti[:], rowmin[:])
        pairs = sb.tile([P, LT * 2], fp32)
        nc.vector.tensor_copy(
            pairs[:].rearrange("p (t two) -> p t two", two=2)[:, :, 0:1],
            tokp1[:].rearrange("p (t one) -> p t one", one=1))
        nc.vector.tensor_copy(
            pairs[:].rearrange("p (t two) -> p t two", two=2)[:, :, 1:2],
            gate_v)

        # ---- scatter (id+1, gate) into per-(shard,expert) regions.
        # Four destination tensors -> four independent 2-link WAW chains
        # instead of one 8-link chain; merged below on the scalar queue
        # (kept off the sync queue, whose w1-stream buffer waits depend
        # transitively on this merge).
        for t in range(LT):
            nc.gpsimd.indirect_dma_start(
                out=igd_ps[t % 4][:], out_offset=bass.IndirectOffsetOnAxis(
                    ap=dsti[:, t:t + 1], axis=0),
                in_=pairs[:, 2 * t:2 * t + 2], in_offset=None,
                bounds_check=RCV - 1, oob_is_err=False)
        mrg = sb.tile([P, RCV * 2 // P], fp32)
        mrgb = sb.tile([P, RCV * 2 // P], fp32)
        mrgc = sb.tile([P, RCV * 2 // P], fp32)
        mrgd = sb.tile([P, RCV * 2 // P], fp32)
        for i, dst in enumerate((mrg, mrgb, mrgc, mrgd)):
            nc.scalar.dma_start(
                dst[:], igd_ps[i][:].rearrange("(p c) two -> p c two", p=P))
        nc.vector.tensor_tensor(
            out=mrg[:], in0=mrg[:], in1=mrgb[:], op=mybir.AluOpType.add)
        nc.vector.tensor_tensor(
            out=mrgc[:], in0=mrgc[:], in1=mrgd[:], op=mybir.AluOpType.add)
        nc.vector.tensor_tensor(
            out=mrg[:], in0=mrg[:], in1=mrgc[:], op=mybir.AluOpType.add)
        nc.scalar.dma_start(
            igd_loc[:].rearrange("(p c) two -> p c two", p=P), mrg[:])

        # ---- the single AllToAll: region e -> core e
        # ---- A set: my own tokens for my expert (local pos < NAS), read
        # straight from the four partial send buffers (each slot is nonzero
        # in exactly one) so the FFN can start without waiting for the
        # merge or the AllToAll.
        pps = []
        for i in range(4):
            pp = sb.tile([P, NA * 2], fp32, name=f"ppA{i}", uniquify=True)
            nc.vector.memset(pp[:], 0.0)
            for c in range(NA):
                nc.gpsimd.indirect_dma_start(
                    out=pp[:, c * 2:(c + 1) * 2], out_offset=None,
                    in_=igd_ps[i][:], in_offset=bass.IndirectOffsetOnAxis(
                        ap=meloct[:, c:c + 1], axis=0),
                    bounds_check=RCV - 1, oob_is_err=False)
            pps.append(pp)
        pairsA = sb.tile([P, NA * 2], fp32)
        nc.vector.tensor_tensor(
            out=pairsA[:], in0=pps[0][:], in1=pps[1][:],
            op=mybir.AluOpType.add)
        nc.vector.tensor_tensor(
            out=pps[2][:], in0=pps[2][:], in1=pps[3][:],
            op=mybir.AluOpType.add)
        nc.vector.tensor_tensor(
            out=pairsA[:], in0=pairsA[:], in1=pps[2][:],
            op=mybir.AluOpType.add)

        nc.gpsimd.collective_compute(
            "AllToAll", mybir.AluOpType.bypass,
            ins=[igd_loc[:]], outs=[igd_rcv[:]],
            replica_groups=[list(range(E))])
        ivA = pairsA[:].rearrange("p (c two) -> p c two", two=2)[:, :, 0:1]
        gvA = pairsA[:].rearrange("p (c two) -> p c two", two=2)[:, :, 1:2]
        vA8 = sb.tile([P, NA], dt.uint8)
        nc.vector.tensor_scalar(
            out=vA8[:], in0=ivA, scalar1=0.5, scalar2=None,
            op0=mybir.AluOpType.is_gt)
        idm1A = sb.tile([P, NA], fp32)
        nc.vector.tensor_scalar_add(
            idm1A[:].rearrange("p (c one) -> p c one", one=1), ivA, -1.0)
        idxfA = sb.tile([P, NA], fp32)
        nc.vector.select(idxfA[:], vA8[:], idm1A[:], bigA[:])
        idxAin = sb.tile([P, NA], dt.int32)
        nc.vector.tensor_copy(idxAin[:], idxfA[:])
        gateA = sb.tile([P, NA], fp32)
        nc.vector.tensor_copy(
            gateA[:].rearrange("p (c one) -> p c one", one=1), gvA)

        w2t = sb.tile([P, DC * M], bf16)
        hT_B = sb.tile([P, DC * NBS], bf16)
        # A's hidden activations alias the first DC*NAS columns of hT_B:
        # A-w2's reads complete exactly when B-w1's writes begin.
        hT_A = hT_B
        dispT_A = sb.tile([P, MC * NAS], bf16)
        dispT_B = sb.tile([P, MC * NBS], bf16)

        with (
            tc.tile_pool(name="psT", bufs=2, space="PSUM") as psT,
            tc.tile_pool(name="psW", bufs=2, space="PSUM") as psW,
            tc.tile_pool(name="ps2", bufs=2, space="PSUM") as ps2,
        ):
            # ---- gather A tokens + transpose into dispT_A
            for c in range(NA):
                gx = sbg.tile([P, M], bf16, tag="gx")
                nc.vector.memset(gx[:], 0.0)
                nc.gpsimd.indirect_dma_start(
                    out=gx[:], out_offset=None, in_=xb[:],
                    in_offset=bass.IndirectOffsetOnAxis(
                        ap=idxAin[:, c:c + 1], axis=0),
                    bounds_check=T - 1, oob_is_err=False)
                for mm in range(MC):
                    ptg = psT.tile([P, P], fp32, tag="ptg")
                    nc.tensor.matmul(
                        ptg[:], lhsT=gx[:, mm * P:(mm + 1) * P],
                        rhs=idb[:], start=True, stop=True)
                    nc.vector.tensor_copy(
                        dispT_A[:, mm * NAS + c * P:mm * NAS + (c + 1) * P],
                        ptg[:])
            # ---- A first layer (w1 stream pass 1)
            for d in range(DC):
                w1t = sbw1.tile([P, M], bf16, tag="w1t")
                nc.sync.dma_start(w1t[:], w1p[d])
                pA = psW.tile([P, NAS], fp32, tag="pA")
                for mc in range(MC):
                    nc.tensor.matmul(
                        pA[:], lhsT=w1t[:, mc * P:(mc + 1) * P],
                        rhs=dispT_A[:, mc * NAS:(mc + 1) * NAS],
                        start=(mc == 0), stop=(mc == MC - 1))
                nc.scalar.activation(
                    hT_A[:, d * NAS:(d + 1) * NAS], pA[:],
                    mybir.ActivationFunctionType.Relu,
                    bias=b1t[:, d:d + 1], scale=1.0)

            # ---- w2 resident load (sync queue: after the A w1 stream)
            for q in range(4):
                nc.sync.dma_start(
                    w2t[:, q * 8 * M:(q + 1) * 8 * M],
                    w2p[:, q * 8:(q + 1) * 8, :])

            # ---- B-prep: counts -> prefix sums -> per-slot src index + pos
            cnt8 = sb.tile([1, E], fp32)
            nc.gpsimd.dma_start(
                cnt8[:].rearrange("p (a s) -> p a s", a=1),
                igd_rcv[:].rearrange("(s cl) two -> two cl s", cl=CL)
                [0:1, 0:1, :])
            mem8 = percs[:, 0:8]
            bvec = percs[:, 8:16]
            r0m1 = percs[:, 16:24]
            cprime = sb.tile([1, E], fp32)
            nc.vector.tensor_scalar(
                out=cprime[:], in0=mem8, scalar1=float(-NAS), scalar2=None,
                op0=mybir.AluOpType.mult)
            nc.vector.tensor_tensor(
                out=cprime[:], in0=cnt8[:], in1=cprime[:],
                op=mybir.AluOpType.add)
            nc.vector.tensor_scalar(
                out=cprime[:], in0=cprime[:], scalar1=0.0, scalar2=None,
                op0=mybir.AluOpType.max)

            def _incl_prefix(src):
                a1 = sb.tile([1, E], fp32)
                nc.vector.tensor_copy(a1[:], src[:])
                nc.vector.tensor_tensor(
                    out=a1[:, 1:8], in0=src[:, 1:8], in1=src[:, 0:7],
                    op=mybir.AluOpType.add)
                a2 = sb.tile([1, E], fp32)
                nc.vector.tensor_copy(a2[:], a1[:])
                nc.vector.tensor_tensor(
                    out=a2[:, 2:8], in0=a1[:, 2:8], in1=a1[:, 0:6],
                    op=mybir.AluOpType.add)
                a3 = sb.tile([1, E], fp32)
                nc.vector.tensor_copy(a3[:], a2[:])
                nc.vector.tensor_tensor(
                    out=a3[:, 4:8], in0=a2[:, 4:8], in1=a2[:, 0:4],
                    op=mybir.AluOpType.add)
                ex_ = sb.tile([1, E], fp32)
                nc.vector.memset(ex_[:], 0.0)
                nc.vector.tensor_copy(ex_[:, 1:8], a3[:, 0:7])
                return ex_

            offx = _incl_prefix(cnt8)     # exclusive prefix of full counts
            boff = _incl_prefix(cprime)   # exclusive prefix of B counts
            boffm = sb.tile([1, E], fp32)
            nc.vector.tensor_scalar_add(boffm[:], boff[:], -0.5)
            srcv = sb.tile([1, E], fp32)
            nc.vector.tensor_tensor(
                out=srcv[:], in0=bvec, in1=boff[:],
                op=mybir.AluOpType.subtract)
            posoffv = sb.tile([1, E], fp32)
            nc.vector.tensor_tensor(
                out=posoffv[:], in0=offx[:], in1=r0m1,
                op=mybir.AluOpType.add)
            nc.vector.tensor_tensor(
                out=posoffv[:], in0=posoffv[:], in1=boff[:],
                op=mybir.AluOpType.subtract)
            offme1 = sb.tile([1, E], fp32)
            nc.vector.tensor_tensor(
                out=offme1[:], in0=mem8, in1=offx[:],
                op=mybir.AluOpType.mult)
            scrt = sb.tile([1, 32], fp32)
            nc.vector.tensor_copy(scrt[:, 0:8], boffm[:])
            nc.vector.tensor_copy(scrt[:, 8:16], srcv[:])
            nc.vector.tensor_copy(scrt[:, 16:24], posoffv[:])
            nc.vector.tensor_reduce(
                out=scrt[:, 24:25].rearrange("p (a s) -> p a s", a=1),
                in_=offme1[:].rearrange("p (a s) -> p a s", a=1),
                axis=mybir.AxisListType.X, op=mybir.AluOpType.add)
            nc.gpsimd.dma_start(scrd[:], scrt[:])
            bct = sb.tile([P, 32], fp32)
            nc.gpsimd.dma_start(bct[:], scrd[:].to_broadcast([P, 32]))

            q3 = qiot[:].rearrange("p (c one) -> p c one", one=1) \
                .to_broadcast([P, NB, E])
            bof3 = bct[:, 0:8].rearrange("p (one s) -> p one s", one=1) \
                .to_broadcast([P, NB, E])
            src3 = bct[:, 8:16].rearrange("p (one s) -> p one s", one=1) \
                .to_broadcast([P, NB, E])
            pos3 = bct[:, 16:24].rearrange("p (one s) -> p one s", one=1) \
                .to_broadcast([P, NB, E])
            m3 = sb.tile([P, NB * E], fp32)
            nc.vector.tensor_tensor(
                out=m3[:].rearrange("p (c s) -> p c s", s=E), in0=q3, in1=bof3,
                op=mybir.AluOpType.is_gt)
            t3 = sb.tile([P, NB * E], fp32)
            nc.vector.tensor_tensor(
                out=t3[:].rearrange("p (c s) -> p c s", s=E),
                in0=m3[:].rearrange("p (c s) -> p c s", s=E), in1=src3,
                op=mybir.AluOpType.mult)
            srcq = sb.tile([P, NB], fp32)
            nc.vector.tensor_reduce(
                out=srcq[:].rearrange("p (c one) -> p c one", one=1),
                in_=t3[:].rearrange("p (c s) -> p c s", s=E),
                axis=mybir.AxisListType.X, op=mybir.AluOpType.max)
            nc.vector.tensor_tensor(
                out=srcq[:], in0=srcq[:], in1=qiot[:],
                op=mybir.AluOpType.add)
            srci = sb.tile([P, NB], dt.int32)
            nc.vector.tensor_copy(srci[:], srcq[:])
            # exact region id from src: s = floor(src / CL)
            sqf = sb.tile([P, NB], fp32)
            nc.vector.tensor_scalar(
                out=sqf[:], in0=srcq[:], scalar1=1.0 / CL, scalar2=None,
                op0=mybir.AluOpType.mult)
            sqi = sb.tile([P, NB], dt.int32)
            nc.vector.tensor_copy(sqi[:], sqf[:])
            nc.vector.tensor_copy(sqf[:], sqi[:])
            m2 = sb.tile([P, NB * E], fp32)
            nc.vector.tensor_tensor(
                out=m2[:].rearrange("p (c s) -> p c s", s=E),
                in0=sqf[:].rearrange("p (c one) -> p c one", one=1)
                .to_broadcast([P, NB, E]),
                in1=siot[:].rearrange("p (one s) -> p one s", one=1)
                .to_broadcast([P, NB, E]),
                op=mybir.AluOpType.is_equal)
            nc.vector.tensor_tensor(
                out=m2[:].rearrange("p (c s) -> p c s", s=E),
                in0=m2[:].rearrange("p (c s) -> p c s", s=E), in1=pos3,
                op=mybir.AluOpType.mult)
            posq = sb.tile([P, NB], fp32)
            nc.vector.tensor_reduce(
                out=posq[:].rearrange("p (c one) -> p c one", one=1),
                in_=m2[:].rearrange("p (c s) -> p c s", s=E),
                axis=mybir.AxisListType.X, op=mybir.AluOpType.max)
            nc.vector.tensor_tensor(
                out=posq[:], in0=posq[:], in1=qiot[:],
                op=mybir.AluOpType.add)

            # ---- compaction gather of (id+1, gate) pairs for the B set
            pairsB = sb.tile([P, NB * 2], fp32)
            nc.vector.memset(pairsB[:], 0.0)
            for c in range(NB):
                nc.gpsimd.indirect_dma_start(
                    out=pairsB[:, c * 2:(c + 1) * 2], out_offset=None,
                    in_=igd_rcv[:], in_offset=bass.IndirectOffsetOnAxis(
                        ap=srci[:, c:c + 1], axis=0),
                    bounds_check=RCV - 1, oob_is_err=False)
            ivB = pairsB[:].rearrange("p (c two) -> p c two", two=2)[:, :, 0:1]
            gvB = pairsB[:].rearrange("p (c two) -> p c two", two=2)[:, :, 1:2]
            vB8 = sb.tile([P, NB], dt.uint8)
            nc.vector.tensor_scalar(
                out=vB8[:], in0=ivB, scalar1=0.5, scalar2=None,
                op0=mybir.AluOpType.is_gt)
            keep8 = sb.tile([P, NB], dt.uint8)
            nc.vector.tensor_scalar(
                out=keep8[:], in0=posq[:], scalar1=float(C) - 0.5, scalar2=None,
                op0=mybir.AluOpType.is_lt)
            nc.vector.tensor_tensor(
                out=vB8[:], in0=vB8[:], in1=keep8[:],
                op=mybir.AluOpType.mult)
            idm1B = sb.tile([P, NB], fp32)
            nc.vector.tensor_scalar_add(
                idm1B[:].rearrange("p (c one) -> p c one", one=1), ivB, -1.0)
            idxfB = sb.tile([P, NB], fp32)
            nc.vector.select(idxfB[:], vB8[:], idm1B[:], bigB[:])
            idxB = sb.tile([P, NB], dt.int32)
            nc.vector.tensor_copy(idxB[:], idxfB[:])
            gateB = sb.tile([P, NB], fp32)
            nc.vector.tensor_copy(
                gateB[:].rearrange("p (c one) -> p c one", one=1), gvB)

            # ---- gather B tokens + transpose into dispT_B
            for c in range(NB):
                gx = sbg.tile([P, M], bf16, tag="gx")
                nc.vector.memset(gx[:], 0.0)
                nc.gpsimd.indirect_dma_start(
                    out=gx[:], out_offset=None, in_=xb[:],
                    in_offset=bass.IndirectOffsetOnAxis(
                        ap=idxB[:, c:c + 1], axis=0),
                    bounds_check=T - 1, oob_is_err=False)
                for mm in range(MC):
                    ptg = psT.tile([P, P], fp32, tag="ptg")
                    nc.tensor.matmul(
                        ptg[:], lhsT=gx[:, mm * P:(mm + 1) * P],
                        rhs=idb[:], start=True, stop=True)
                    nc.vector.tensor_copy(
                        dispT_B[:, mm * NBS + c * P:mm * NBS + (c + 1) * P],
                        ptg[:])

            # ---- A-drop mask (uses off_me, available post-A2A) + A second
            # layer + output scatter
            posA = sb.tile([P, NA], fp32)
            nc.vector.tensor_tensor(
                out=posA[:].rearrange("p (c one) -> p c one", one=1),
                in0=qiot[:, 0:NA].rearrange("p (c one) -> p c one", one=1),
                in1=bct[:, 24:25].rearrange("p (c one) -> p c one", one=1)
                .to_broadcast([P, NA, 1]),
                op=mybir.AluOpType.add)
            keepA = sb.tile([P, NA], dt.uint8)
            nc.vector.tensor_scalar(
                out=keepA[:], in0=posA[:], scalar1=float(C) - 0.5, scalar2=None,
                op0=mybir.AluOpType.is_lt)
            nc.vector.tensor_tensor(
                out=keepA[:], in0=keepA[:], in1=vA8[:],
                op=mybir.AluOpType.mult)
            idxfAo = sb.tile([P, NA], fp32)
            nc.vector.select(idxfAo[:], keepA[:], idm1A[:], bigA[:])
            idxAo = sb.tile([P, NA], dt.int32)
            nc.vector.tensor_copy(idxAo[:], idxfAo[:])

            for s5 in range(NA):
                po0 = ps2.tile([P, 512], fp32, tag="po")
                po1 = ps2.tile([P, 512], fp32, tag="po")
                for d in range(DC):
                    lhs = hT_A[:, d * NAS + s5 * P:d * NAS + (s5 + 1) * P]
                    nc.tensor.matmul(
                        po0[:], lhsT=lhs, rhs=w2t[:, d * M:d * M + 512],
                        start=(d == 0), stop=(d == DC - 1))
                    nc.tensor.matmul(
                        po1[:], lhsT=lhs, rhs=w2t[:, d * M + 512:(d + 1) * M],
                        start=(d == 0), stop=(d == DC - 1))
                st = sbst.tile([P, M], fp32, tag="st")
                for hh, po in ((0, po0), (1, po1)):
                    nc.vector.tensor_tensor(
                        out=st[:, hh * 512:(hh + 1) * 512], in0=po[:],
                        in1=b2t[:, hh * 512:(hh + 1) * 512],
                        op=mybir.AluOpType.add)
                nc.vector.tensor_scalar_mul(
                    st[:], st[:], gateA[:, s5:s5 + 1])
                nc.gpsimd.indirect_dma_start(
                    out=outd[:], out_offset=bass.IndirectOffsetOnAxis(
                        ap=idxAo[:, s5:s5 + 1], axis=0),
                    in_=st[:], in_offset=None,
                    bounds_check=T - 1, oob_is_err=False)

            # ---- B first layer (w1 stream pass 2)
            for d in range(DC):
                w1t = sbw1.tile([P, M], bf16, tag="w1t")
                nc.sync.dma_start(w1t[:], w1p[d])
                pA = psW.tile([P, 512], fp32, tag="pA")
                pB = psW.tile([P, 512], fp32, tag="pB")
                for mc in range(MC):
                    lhs = w1t[:, mc * P:(mc + 1) * P]
                    nc.tensor.matmul(
                        pA[:], lhsT=lhs,
                        rhs=dispT_B[:, mc * NBS:mc * NBS + 512],
                        start=(mc == 0), stop=(mc == MC - 1))
                    nc.tensor.matmul(
                        pB[:], lhsT=lhs,
                        rhs=dispT_B[:, mc * NBS + 512:(mc + 1) * NBS],
                        start=(mc == 0), stop=(mc == MC - 1))
                nc.scalar.activation(
                    hT_B[:, d * NBS:d * NBS + 512], pA[:],
                    mybir.ActivationFunctionType.Relu,
                    bias=b1t[:, d:d + 1], scale=1.0)
                nc.scalar.activation(
                    hT_B[:, d * NBS + 512:(d + 1) * NBS], pB[:],
                    mybir.ActivationFunctionType.Relu,
                    bias=b1t[:, d:d + 1], scale=1.0)

            # ---- B second layer + output scatter
            for s5 in range(NB):
                po0 = ps2.tile([P, 512], fp32, tag="po")
                po1 = ps2.tile([P, 512], fp32, tag="po")
                for d in range(DC):
                    lhs = hT_B[:, d * NBS + s5 * P:d * NBS + (s5 + 1) * P]
                    nc.tensor.matmul(
                        po0[:], lhsT=lhs, rhs=w2t[:, d * M:d * M + 512],
                        start=(d == 0), stop=(d == DC - 1))
                    nc.tensor.matmul(
                        po1[:], lhsT=lhs, rhs=w2t[:, d * M + 512:(d + 1) * M],
                        start=(d == 0), stop=(d == DC - 1))
                st = sbst.tile([P, M], fp32, tag="st")
                for hh, po in ((0, po0), (1, po1)):
                    nc.vector.tensor_tensor(
                        out=st[:, hh * 512:(hh + 1) * 512], in0=po[:],
                        in1=b2t[:, hh * 512:(hh + 1) * 512],
                        op=mybir.AluOpType.add)
                nc.vector.tensor_scalar_mul(
                    st[:], st[:], gateB[:, s5:s5 + 1])
                nc.gpsimd.indirect_dma_start(
                    out=outd[:], out_offset=bass.IndirectOffsetOnAxis(
                        ap=idxB[:, s5:s5 + 1], axis=0),
                    in_=st[:], in_offset=None,
                    bounds_check=T - 1, oob_is_err=False)

    nc.compile()
    return nc


def _make_w64():
    w = np.zeros((LE, LE + E), dtype=np.float32)
    for tip in range(LT):
        for ep in range(E):
            r = tip * E + ep
            for ti in range(LT):
                if tip < ti:
                    w[r, ti * E + ep] = 1.0
            w[r, LE + ep] = 1.0
    return w


def _prep_inputs(x, wg, w1, b1, w2, b2):
    bf = ml_dtypes.bfloat16
    tokens = np.ascontiguousarray(x.reshape(T, M)).astype(np.float32)
    xT = np.ascontiguousarray(tokens.T)
    xb = tokens.astype(bf)
    wgf = np.ascontiguousarray(wg.astype(np.float32))
    eiota = np.tile(np.arange(E, dtype=np.float32), LT)[None, :].repeat(P, 0)
    triu = np.triu(np.ones((P, P), dtype=np.float32))
    identf = np.eye(P, dtype=np.float32)
    identb = np.eye(P).astype(bf)
    w64 = _make_w64()
    ecl = np.tile(np.arange(E, dtype=np.float32) * CL, LT)[None, :].repeat(P, 0)
    qiota = (np.arange(NB, dtype=np.float32)[None, :] * P
             + np.arange(P, dtype=np.float32)[:, None]).copy()
    siota = np.arange(E, dtype=np.float32)[None, :].repeat(P, 0)
    cpak = np.concatenate(
        [eiota, triu, identf, ecl,
         np.zeros((P, 8), np.float32), qiota, siota], axis=1)
    in_maps = []
    for e in range(E):
        w1e = np.ascontiguousarray(w1[e]).astype(bf)
        w1pk = np.ascontiguousarray(
            w1e.reshape(MC, P, DC, P).transpose(2, 1, 0, 3))
        w2e = np.ascontiguousarray(w2[e]).astype(bf)
        w2pk = np.ascontiguousarray(
            w2e.reshape(DC, P, M).transpose(1, 0, 2))
        tokp1 = (e * TSH + np.arange(TSH, dtype=np.float32)
                 .reshape(LT, P).T + 1.0).copy()
        cpk = cpak.copy()
        cpk[:, 384:392] = tokp1
        meloc = (e * CL + 1 + qiota[:, :NA]).astype(np.int32)
        perc = np.zeros((1, 24), dtype=np.float32)
        for s in range(E):
            perc[0, s] = 1.0 if s == e else 0.0
            perc[0, 8 + s] = s * CL + (NAS + 1 if s == e else 1)
            perc[0, 16 + s] = float(NAS) if s == e else 0.0
        in_maps.append({
            "xTs": np.ascontiguousarray(xT[:, e * TSH:(e + 1) * TSH]),
            "xb": xb, "wg": wgf,
            "w1p": w1pk, "w2p": w2pk,
            "b1v": np.ascontiguousarray(b1[e]).astype(np.float32),
            "b2b": np.tile(np.asarray(b2[e], dtype=np.float32), (P, 1)),
            "cpakd": np.ascontiguousarray(cpk),
            "identb": identb, "w64d": w64,
            "melocd": meloc, "percd": perc,
        })
    return in_maps


def kernel(x, wg, w1, b1, w2, b2, _trace=False):
    if "nc" not in _CACHE:
        _CACHE["nc"] = _build_nc()
    nc = _CACHE["nc"]
    in_maps = _prep_inputs(
        np.asarray(x), np.asarray(wg), np.asarray(w1),
        np.asarray(b1), np.asarray(w2), np.asarray(b2))
    res = run_bass_kernel_spmd(nc, in_maps, list(range(E)), trace=_trace)
    _CACHE["last_results"] = res
    full = np.zeros((T, M), dtype=np.float32)
    for e in range(E):
        full += res.results[e]["out"]
    return full.reshape(B, S, M)


# revision 32
# speedup vs baseline: 1.0203x; 1.0160x over previous
"""MoE top-1 routing kernel for 8 TRN2 NeuronCores (expert parallelism).

Self-contained: takes full inputs, shards experts across 8 cores, returns the
full output (host sums the 8 disjoint per-expert partials).

v5 design (local-first pipelining, single collective):
- Gating is token-sharded: each core computes fp32 logits for its own 1024
  tokens, then DVE softmax/argmax and local (shard-internal) queue positions
  via one triangular matmul. No counts-AllGather is needed: each shard
  scatters (token_id+1, gate) into per-(shard,expert) regions of a [E*CL,2]
  send buffer at LOCAL positions (CL=512 rows/region, row 0 = count), and one
  32 KB AllToAll delivers region e to core e.
- Latency hiding: tokens of a core's OWN shard choosing its OWN expert (the
  "A set", local pos < 256) are known before the AllToAll; the core gathers
  them from the replicated token buffer and starts the FFN on them (2 chunks
  of 128 slots) while the collective + receive-side compaction for the
  remaining "B set" (8 chunks) is still in flight. Queue order is irrelevant
  to the math: outputs are scattered back by token id, and capacity drops
  (global pos >= C) are applied exactly at output-scatter time using the
  received per-shard counts.
- Receive-side compaction is a computed gather: per-shard counts (embedded at
  region row 0) -> prefix sums on DVE -> per-slot source index + global
  position -> one level of 8B-row indirect gathers, then the usual 2KB-row
  token gathers from the bf16 token buffer.
- FFN in bf16: w2 resident in SBUF (8 MB), w1 streamed once per pass (A then
  B), fused bias+ReLU on the scalar engine, gate-scaled rows scattered into
  the output by token id.
"""
import numpy as np
import ml_dtypes
from contextlib import ExitStack

import concourse.bass as bass
import concourse.tile as tile
from concourse import bacc, mybir
from concourse.bass_utils import run_bass_kernel_spmd

dt = mybir.dt

B, S, M, E, DFF = 4, 2048, 1024, 8, 4096
T = B * S
C = int(1.25 * T / E)          # 1280 capacity per expert
P = 128
MC = M // P                    # 8
DC = DFF // P                  # 32
TSH = T // E                   # 1024 tokens per shard
LT = TSH // P                  # 8
LE = LT * E                    # 64
CL = 512                       # rows per (shard, expert) region (row0=count)
RCV = E * CL                   # 4096
NA = 2                         # local-first chunks (256 slots)
NAS = NA * P
NB = 8                         # remote/compacted chunks (1024 slots)
NBS = NB * P
BIG = 1.5e9

_CACHE = {}


def _build_nc():
    nc = bacc.Bacc("TRN2", target_bir_lowering=False, debug=False)

    xTs = nc.dram_tensor("xTs", [M, TSH], dt.float32, kind="ExternalInput")
    xb = nc.dram_tensor("xb", [T, M], dt.bfloat16, kind="ExternalInput")
    wg = nc.dram_tensor("wg", [M, E], dt.float32, kind="ExternalInput")
    w1p = nc.dram_tensor("w1p", [DC, P, MC, P], dt.bfloat16, kind="ExternalInput")
    w2p = nc.dram_tensor("w2p", [P, DC, M], dt.bfloat16, kind="ExternalInput")
    b1v = nc.dram_tensor("b1v", [DFF], dt.float32, kind="ExternalInput")
    b2b = nc.dram_tensor("b2b", [P, M], dt.float32, kind="ExternalInput")
    cpakd = nc.dram_tensor("cpakd", [P, 408], dt.float32, kind="ExternalInput")
    identb = nc.dram_tensor("identb", [P, P], dt.bfloat16, kind="ExternalInput")
    w64d = nc.dram_tensor("w64d", [LE, LE + E], dt.float32, kind="ExternalInput")
    melocd = nc.dram_tensor("melocd", [P, NA], dt.int32, kind="ExternalInput")
    percd = nc.dram_tensor("percd", [1, 24], dt.float32, kind="ExternalInput")
    outd = nc.dram_tensor("out", [T, M], dt.float32, kind="ExternalOutput")

    igd_loc = nc.dram_tensor("igd_loc", [RCV, 2], dt.float32)
    igd_ps = [nc.dram_tensor(f"igd_p{i}", [RCV, 2], dt.float32)
              for i in range(4)]
    igd_rcv = nc.dram_tensor("igd_rcv", [RCV, 2], dt.float32)
    scrd = nc.dram_tensor("scrd", [1, 32], dt.float32)
    wrm_l = nc.dram_tensor("wrm_l", [8, 2], dt.float32)
    wrm_a = nc.dram_tensor("wrm_a", [64, 2], dt.float32, addr_space="Shared")

    fp32 = dt.float32
    bf16 = dt.bfloat16

    with tile.TileContext(nc) as tc, ExitStack() as ctx:
        sb = ctx.enter_context(tc.tile_pool(name="sb", bufs=1))
        sbx = ctx.enter_context(tc.tile_pool(name="sbx", bufs=6))
        sbw1 = ctx.enter_context(tc.tile_pool(name="sbw1", bufs=4))
        sbg = ctx.enter_context(tc.tile_pool(name="sbg", bufs=4))
        sbst = ctx.enter_context(tc.tile_pool(name="sbst", bufs=2))

        # ---- warmup collective: absorbs CC channel init + start barrier
        wz = sb.tile([8, 2], fp32)
        nc.vector.memset(wz[:], 0.0)
        nc.sync.dma_start(wrm_l[:], wz[:])
        nc.gpsimd.collective_compute(
            "AllGather", mybir.AluOpType.bypass,
            ins=[wrm_l[:]], outs=[wrm_a[:]],
            replica_groups=[list(range(E))])

        # ---- const loads (sync queue; xTs chunks follow in the gating loop)
        wgt = sb.tile([P, MC * E], fp32)
        nc.sync.dma_start(wgt[:], wg[:].rearrange("(mc p) e -> p mc e", p=P))
        cpak = sb.tile([P, 408], fp32)
        nc.sync.dma_start(cpak[:], cpakd[:])
        eit = cpak[:, 0:64]
        trit = cpak[:, 64:192]
        idf = cpak[:, 192:320]
        eclt = cpak[:, 320:384]
        tokp1 = cpak[:, 384:392]
        qiot = cpak[:, 392:400]
        siot = cpak[:, 400:408]
        idb = sb.tile([P, P], bf16)
        nc.sync.dma_start(idb[:], identb[:])
        w64t = sb.tile([LE, LE + E], fp32)
        nc.sync.dma_start(w64t[:], w64d[:])
        meloct = sb.tile([P, NA], dt.int32)
        nc.sync.dma_start(meloct[:], melocd[:])
        percs = sb.tile([1, 24], fp32)
        nc.sync.dma_start(percs[:], percd[:])
        b1t = sb.tile([P, DC], fp32)
        nc.sync.dma_start(b1t[:], b1v[:].rearrange("(d p) -> p d", p=P))
        # prefetch the first w1 chunks before gating so the A first-layer
        # stream is not scheduled behind the routing vector chain
        w1pre = []
        for dd in range(4):
            w1t = sbw1.tile([P, M], bf16, tag="w1t", name=f"w1pre{dd}",
                            uniquify=True)
            nc.sync.dma_start(w1t[:], w1p[dd])
            w1pre.append(w1t)
        # PE warm-up: keep the HAM activity window busy before the logits MMs
        wrmj = sb.tile([P, P], bf16)
        nc.vector.memset(wrmj[:], 0.0)
        # scalar queue: b2 broadcast + zero-prefill of the send buffer
        b2t = sb.tile([P, M], fp32)
        nc.scalar.dma_start(b2t[:], b2b[:])
        zpre = sb.tile([P, RCV * 2 // P], fp32)
        nc.vector.memset(zpre[:], 0.0)
        for i in range(4):
            nc.scalar.dma_start(
                igd_ps[i][:].rearrange("(p c) two -> p c two", p=P), zpre[:])

        ones1 = sb.tile([1, P], fp32)
        nc.vector.memset(ones1[:], 1.0)
        onescol = sb.tile([P, 1], fp32)
        nc.vector.memset(onescol[:], 1.0)
        nines = sb.tile([P, LE], fp32)
        nc.vector.memset(nines[:], 9.0)
        huget = sb.tile([P, LE], fp32)
        nc.vector.memset(huget[:], BIG)
        bigA = sb.tile([P, NA], fp32)
        nc.vector.memset(bigA[:], BIG)
        bigB = sb.tile([P, NB], fp32)
        nc.vector.memset(bigB[:], BIG)

        # ---- gating: fp32 logits for my 1024 tokens
        eg_stk = sb.tile([P, LT * 2], fp32)
        lg_stk = sb.tile([P, LE], fp32)
        lgT = sb.tile([8, TSH], fp32)
        with tc.tile_pool(name="psg", bufs=2, space="PSUM") as psg:
            pwrm = psg.tile([P, P], fp32, tag="pwrm")
            for _w in range(24):
                nc.tensor.matmul(pwrm[:], lhsT=wrmj[:], rhs=idb[:],
                                 start=(_w == 0), stop=(_w == 23))
            pl0 = psg.tile([8, 512], fp32, tag="pl0")
            pl1 = psg.tile([8, 512], fp32, tag="pl1")
            for k in range(MC):
                xt0 = sbx.tile([P, 512], fp32, tag="xt")
                nc.sync.dma_start(xt0[:], xTs[k * P:(k + 1) * P, 0:512])
                nc.tensor.matmul(
                    pl0[:], lhsT=wgt[:, k * E:(k + 1) * E], rhs=xt0[:],
                    start=(k == 0), stop=(k == MC - 1))
                xt1 = sbx.tile([P, 512], fp32, tag="xt")
                nc.sync.dma_start(xt1[:], xTs[k * P:(k + 1) * P, 512:1024])
                nc.tensor.matmul(
                    pl1[:], lhsT=wgt[:, k * E:(k + 1) * E], rhs=xt1[:],
                    start=(k == 0), stop=(k == MC - 1))
            nc.vector.tensor_copy(lgT[:, 0:512], pl0[:])
            nc.vector.tensor_copy(lgT[:, 512:1024], pl1[:])
            for ti in range(LT):
                pq = psg.tile([P, E], fp32, tag="pq")
                nc.tensor.transpose(
                    out=pq[:], in_=lgT[:, ti * P:(ti + 1) * P],
                    identity=idf[:8, :8])
                nc.vector.tensor_copy(lg_stk[:, ti * E:(ti + 1) * E], pq[:])
        lg3 = lg_stk[:].rearrange("p (ti e) -> p ti e", e=E)
        mx_stk = sb.tile([P, LT], fp32)
        nc.vector.tensor_reduce(
            out=mx_stk[:], in_=lg3, axis=mybir.AxisListType.X,
            op=mybir.AluOpType.max)
        mxb = mx_stk[:].rearrange("p (ti one) -> p ti one", one=1).to_broadcast([P, LT, E])
        ls = sb.tile([P, LE], fp32)
        nc.vector.tensor_tensor(
            out=ls[:].rearrange("p (ti e) -> p ti e", e=E), in0=lg3, in1=mxb,
            op=mybir.AluOpType.subtract)
        ex = sb.tile([P, LE], fp32)
        nc.scalar.activation(ex[:], ls[:], mybir.ActivationFunctionType.Exp)
        s_stk = sb.tile([P, LT], fp32)
        nc.vector.tensor_reduce(
            out=s_stk[:], in_=ex[:].rearrange("p (ti e) -> p ti e", e=E),
            axis=mybir.AxisListType.X, op=mybir.AluOpType.add)
        nc.vector.reciprocal(
            eg_stk[:].rearrange("p (ti two) -> p ti two", two=2)[:, :, 1:2],
            s_stk[:].rearrange("p (ti one) -> p ti one", one=1))
        oh = sb.tile([P, LE], dt.uint8)
        nc.vector.tensor_tensor(
            out=oh[:].rearrange("p (ti e) -> p ti e", e=E), in0=lg3, in1=mxb,
            op=mybir.AluOpType.is_equal)
        msk = sb.tile([P, LE], fp32)
        nc.vector.select(msk[:], oh[:], eit[:], nines[:])
        nc.vector.tensor_reduce(
            out=eg_stk[:].rearrange("p (ti two) -> p ti two", two=2)[:, :, 0:1],
            in_=msk[:].rearrange("p (ti e) -> p ti e", e=E),
            axis=mybir.AxisListType.X, op=mybir.AluOpType.min)
        eidx_v = eg_stk[:].rearrange("p (ti two) -> p ti two", two=2)[:, :, 0:1]
        gate_v = eg_stk[:].rearrange("p (ti two) -> p ti two", two=2)[:, :, 1:2]
        mine_all = sb.tile([P, LE], fp32)
        nc.vector.tensor_tensor(
            out=mine_all[:].rearrange("p (ti e) -> p ti e", e=E),
            in0=eidx_v.to_broadcast([P, LT, E]),
            in1=eit[:].rearrange("p (ti e) -> p ti e", e=E),
            op=mybir.AluOpType.is_equal)

        # ---- local queue positions + per-expert counts (all shard-local)
        offsb = sb.tile([1, LE + E], fp32)
        palls = sb.tile([P, LE], fp32)
        with tc.tile_pool(name="ppb", bufs=1, space="PSUM") as ppb:
            pts = ppb.tile([LE, 1], fp32, tag="pts")
            nc.tensor.matmul(pts[:], lhsT=mine_all[:], rhs=onescol[:],
                             start=True, stop=True)
            tscol = sb.tile([LE, 1], fp32)
            nc.vector.tensor_copy(tscol[:], pts[:])
            poffs = ppb.tile([1, LE + E], fp32, tag="poffs")
            nc.tensor.matmul(poffs[:], lhsT=tscol[:], rhs=w64t[:],
                             start=True, stop=True)
            nc.vector.tensor_copy(offsb[:], poffs[:])
            # counts c_{me,e} -> row 0 of each region of the send buffer
            nc.sync.dma_start(
                igd_ps[0][:].rearrange("(e cl) two -> cl e two", cl=CL)
                [0:1, :, 0:1],
                offsb[:, LE:LE + E].rearrange("p (e one) -> p e one", one=1))
            pall = ppb.tile([P, LE], fp32, tag="pall")
            nc.tensor.matmul(pall[:], lhsT=trit[:], rhs=mine_all[:],
                             start=True, stop=False)
            nc.tensor.matmul(pall[:], lhsT=ones1[:], rhs=offsb[:, 0:LE],
                             start=False, stop=True)
            nc.vector.tensor_copy(palls[:], pall[:])
        mu8 = sb.tile([P, LE], dt.uint8)
        nc.vector.tensor_scalar(
            out=mu8[:], in0=mine_all[:], scalar1=0.5, scalar2=None,
            op0=mybir.AluOpType.is_gt)
        cu8 = sb.tile([P, LE], dt.uint8)
        nc.vector.tensor_scalar(
            out=cu8[:], in0=palls[:], scalar1=float(CL) - 0.5, scalar2=None,
            op0=mybir.AluOpType.is_lt)
        au8 = sb.tile([P, LE], dt.uint8)
        nc.vector.tensor_tensor(
            out=au8[:], in0=mu8[:], in1=cu8[:], op=mybir.AluOpType.mult)
        s1 = sb.tile([P, LE], fp32)
        nc.vector.select(s1[:], au8[:], palls[:], huget[:])
        dstf = sb.tile([P, LE], fp32)
        nc.vector.tensor_tensor(
            out=dstf[:], in0=s1[:], in1=eclt[:], op=mybir.AluOpType.add)
        rowmin = sb.tile([P, LT], fp32)
        nc.vector.tensor_reduce(
            out=rowmin[:].rearrange("p (ti one) -> p ti one", one=1),
            in_=dstf[:].rearrange("p (ti e) -> p ti e", e=E),
            axis=mybir.AxisListType.X, op=mybir.AluOpType.min)
        dsti = sb.tile([P, LT], dt.int32)
        nc.vector.tensor_copy(dsti[:], rowmin[:])
        pairs = sb.tile([P, LT * 2], fp32)
        nc.vector.tensor_copy(
            pairs[:].rearrange("p (t two) -> p t two", two=2)[:, :, 0:1],
            tokp1[:].rearrange("p (t one) -> p t one", one=1))
        nc.vector.tensor_copy(
            pairs[:].rearrange("p (t two) -> p t two", two=2)[:, :, 1:2],
            gate_v)

        # ---- scatter (id+1, gate) into per-(shard,expert) regions.
        # Four destination tensors -> four independent 2-link WAW chains
        # instead of one 8-link chain; merged below on the scalar queue
        # (kept off the sync queue, whose w1-stream buffer waits depend
        # transitively on this merge).
        for t in range(LT):
            nc.gpsimd.indirect_dma_start(
                out=igd_ps[t % 4][:], out_offset=bass.IndirectOffsetOnAxis(
                    ap=dsti[:, t:t + 1], axis=0),
                in_=pairs[:, 2 * t:2 * t + 2], in_offset=None,
                bounds_check=RCV - 1, oob_is_err=False)
        mrg = sb.tile([P, RCV * 2 // P], fp32)
        mrgb = sb.tile([P, RCV * 2 // P], fp32)
        mrgc = sb.tile([P, RCV * 2 // P], fp32)
        mrgd = sb.tile([P, RCV * 2 // P], fp32)
        for i, dst in enumerate((mrg, mrgb, mrgc, mrgd)):
            nc.scalar.dma_start(
                dst[:], igd_ps[i][:].rearrange("(p c) two -> p c two", p=P))
        nc.vector.tensor_tensor(
            out=mrg[:], in0=mrg[:], in1=mrgb[:], op=mybir.AluOpType.add)
        nc.vector.tensor_tensor(
            out=mrgc[:], in0=mrgc[:], in1=mrgd[:], op=mybir.AluOpType.add)
        nc.vector.tensor_tensor(
            out=mrg[:], in0=mrg[:], in1=mrgc[:], op=mybir.AluOpType.add)
        nc.scalar.dma_start(
            igd_loc[:].rearrange("(p c) two -> p c two", p=P), mrg[:])

        # ---- the single AllToAll: region e -> core e
        # ---- A set: my own tokens for my expert (local pos < NAS), read
        # straight from the four partial send buffers (each slot is nonzero
        # in exactly one) so the FFN can start without waiting for the
        # merge or the AllToAll.
        pps = []
        for i in range(4):
            pp = sb.tile([P, NA * 2], fp32, name=f"ppA{i}", uniquify=True)
            nc.vector.memset(pp[:], 0.0)
            for c in range(NA):
                nc.gpsimd.indirect_dma_start(
                    out=pp[:, c * 2:(c + 1) * 2], out_offset=None,
                    in_=igd_ps[i][:], in_offset=bass.IndirectOffsetOnAxis(
                        ap=meloct[:, c:c + 1], axis=0),
                    bounds_check=RCV - 1, oob_is_err=False)
            pps.append(pp)
        pairsA = sb.tile([P, NA * 2], fp32)
        nc.vector.tensor_tensor(
            out=pairsA[:], in0=pps[0][:], in1=pps[1][:],
            op=mybir.AluOpType.add)
        nc.vector.tensor_tensor(
            out=pps[2][:], in0=pps[2][:], in1=pps[3][:],
            op=mybir.AluOpType.add)
        nc.vector.tensor_tensor(
            out=pairsA[:], in0=pairsA[:], in1=pps[2][:],
            op=mybir.AluOpType.add)

        nc.gpsimd.collective_compute(
            "AllToAll", mybir.AluOpType.bypass,
            ins=[igd_loc[:]], outs=[igd_rcv[:]],
            replica_groups=[list(range(E))])
        ivA = pairsA[:].rearrange("p (c two) -> p c two", two=2)[:, :, 0:1]
        gvA = pairsA[:].rearrange("p (c two) -> p c two", two=2)[:, :, 1:2]
        vA8 = sb.tile([P, NA], dt.uint8)
        nc.vector.tensor_scalar(
            out=vA8[:], in0=ivA, scalar1=0.5, scalar2=None,
            op0=mybir.AluOpType.is_gt)
        idm1A = sb.tile([P, NA], fp32)
        nc.vector.tensor_scalar_add(
            idm1A[:].rearrange("p (c one) -> p c one", one=1), ivA, -1.0)
        idxfA = sb.tile([P, NA], fp32)
        nc.vector.select(idxfA[:], vA8[:], idm1A[:], bigA[:])
        idxAin = sb.tile([P, NA], dt.int32)
        nc.vector.tensor_copy(idxAin[:], idxfA[:])
        gateA = sb.tile([P, NA], fp32)
        nc.vector.tensor_copy(
            gateA[:].rearrange("p (c one) -> p c one", one=1), gvA)

        w2t = sb.tile([P, DC * M], bf16)
        hT_B = sb.tile([P, DC * NBS], bf16)
        # A's hidden activations alias the first DC*NAS columns of hT_B:
        # A-w2's reads complete exactly when B-w1's writes begin.
        hT_A = hT_B
        dispT_A = sb.tile([P, MC * NAS], bf16)
        dispT_B = sb.tile([P, MC * NBS], bf16)

        with (
            tc.tile_pool(name="psT", bufs=2, space="PSUM") as psT,
            tc.tile_pool(name="psW", bufs=2, space="PSUM") as psW,
            tc.tile_pool(name="ps2", bufs=2, space="PSUM") as ps2,
        ):
            # ---- gather A tokens + transpose into dispT_A
            for c in range(NA):
                gx = sbg.tile([P, M], bf16, tag="gx")
                nc.vector.memset(gx[:], 0.0)
                nc.gpsimd.indirect_dma_start(
                    out=gx[:], out_offset=None, in_=xb[:],
                    in_offset=bass.IndirectOffsetOnAxis(
                        ap=idxAin[:, c:c + 1], axis=0),
                    bounds_check=T - 1, oob_is_err=False)
                for mm in range(MC):
                    ptg = psT.tile([P, P], fp32, tag="ptg")
                    nc.tensor.matmul(
                        ptg[:], lhsT=gx[:, mm * P:(mm + 1) * P],
                        rhs=idb[:], start=True, stop=True)
                    nc.vector.tensor_copy(
                        dispT_A[:, mm * NAS + c * P:mm * NAS + (c + 1) * P],
                        ptg[:])
            # ---- A first layer (w1 stream pass 1)
            for d in range(DC):
                if d < 4:
                    w1t = w1pre[d]
                else:
                    w1t = sbw1.tile([P, M], bf16, tag="w1t")
                    nc.sync.dma_start(w1t[:], w1p[d])
                pA = psW.tile([P, NAS], fp32, tag="pA")
                for mc in range(MC):
                    nc.tensor.matmul(
                        pA[:], lhsT=w1t[:, mc * P:(mc + 1) * P],
                        rhs=dispT_A[:, mc * NAS:(mc + 1) * NAS],
                        start=(mc == 0), stop=(mc == MC - 1))
                nc.scalar.activation(
                    hT_A[:, d * NAS:(d + 1) * NAS], pA[:],
                    mybir.ActivationFunctionType.Relu,
                    bias=b1t[:, d:d + 1], scale=1.0)

            # ---- w2 resident load (sync queue: after the A w1 stream)
            for q in range(4):
                nc.sync.dma_start(
                    w2t[:, q * 8 * M:(q + 1) * 8 * M],
                    w2p[:, q * 8:(q + 1) * 8, :])

            # ---- B-prep: counts -> prefix sums -> per-slot src index + pos
            cnt8 = sb.tile([1, E], fp32)
            nc.gpsimd.dma_start(
                cnt8[:].rearrange("p (a s) -> p a s", a=1),
                igd_rcv[:].rearrange("(s cl) two -> two cl s", cl=CL)
                [0:1, 0:1, :])
            mem8 = percs[:, 0:8]
            bvec = percs[:, 8:16]
            r0m1 = percs[:, 16:24]
            cprime = sb.tile([1, E], fp32)
            nc.vector.tensor_scalar(
                out=cprime[:], in0=mem8, scalar1=float(-NAS), scalar2=None,
                op0=mybir.AluOpType.mult)
            nc.vector.tensor_tensor(
                out=cprime[:], in0=cnt8[:], in1=cprime[:],
                op=mybir.AluOpType.add)
            nc.vector.tensor_scalar(
                out=cprime[:], in0=cprime[:], scalar1=0.0, scalar2=None,
                op0=mybir.AluOpType.max)

            def _incl_prefix(src):
                a1 = sb.tile([1, E], fp32)
                nc.vector.tensor_copy(a1[:], src[:])
                nc.vector.tensor_tensor(
                    out=a1[:, 1:8], in0=src[:, 1:8], in1=src[:, 0:7],
                    op=mybir.AluOpType.add)
                a2 = sb.tile([1, E], fp32)
                nc.vector.tensor_copy(a2[:], a1[:])
                nc.vector.tensor_tensor(
                    out=a2[:, 2:8], in0=a1[:, 2:8], in1=a1[:, 0:6],
                    op=mybir.AluOpType.add)
                a3 = sb.tile([1, E], fp32)
                nc.vector.tensor_copy(a3[:], a2[:])
                nc.vector.tensor_tensor(
                    out=a3[:, 4:8], in0=a2[:, 4:8], in1=a2[:, 0:4],
                    op=mybir.AluOpType.add)
                ex_ = sb.tile([1, E], fp32)
                nc.vector.memset(ex_[:], 0.0)
                nc.vector.tensor_copy(ex_[:, 1:8], a3[:, 0:7])
                return ex_

            offx = _incl_prefix(cnt8)     # exclusive prefix of full counts
            boff = _incl_prefix(cprime)   # exclusive prefix of B counts
            boffm = sb.tile([1, E], fp32)
            nc.vector.tensor_scalar_add(boffm[:], boff[:], -0.5)
            srcv = sb.tile([1, E], fp32)
            nc.vector.tensor_tensor(
                out=srcv[:], in0=bvec, in1=boff[:],
                op=mybir.AluOpType.subtract)
            posoffv = sb.tile([1, E], fp32)
            nc.vector.tensor_tensor(
                out=posoffv[:], in0=offx[:], in1=r0m1,
                op=mybir.AluOpType.add)
            nc.vector.tensor_tensor(
                out=posoffv[:], in0=posoffv[:], in1=boff[:],
                op=mybir.AluOpType.subtract)
            offme1 = sb.tile([1, E], fp32)
            nc.vector.tensor_tensor(
                out=offme1[:], in0=mem8, in1=offx[:],
                op=mybir.AluOpType.mult)
            scrt = sb.tile([1, 32], fp32)
            nc.vector.tensor_copy(scrt[:, 0:8], boffm[:])
            nc.vector.tensor_copy(scrt[:, 8:16], srcv[:])
            nc.vector.tensor_copy(scrt[:, 16:24], posoffv[:])
            nc.vector.tensor_reduce(
                out=scrt[:, 24:25].rearrange("p (a s) -> p a s", a=1),
                in_=offme1[:].rearrange("p (a s) -> p a s", a=1),
                axis=mybir.AxisListType.X, op=mybir.AluOpType.add)
            nc.gpsimd.dma_start(scrd[:], scrt[:])
            bct = sb.tile([P, 32], fp32)
            nc.gpsimd.dma_start(bct[:], scrd[:].to_broadcast([P, 32]))

            q3 = qiot[:].rearrange("p (c one) -> p c one", one=1) \
                .to_broadcast([P, NB, E])
            bof3 = bct[:, 0:8].rearrange("p (one s) -> p one s", one=1) \
                .to_broadcast([P, NB, E])
            src3 = bct[:, 8:16].rearrange("p (one s) -> p one s", one=1) \
                .to_broadcast([P, NB, E])
            pos3 = bct[:, 16:24].rearrange("p (one s) -> p one s", one=1) \
                .to_broadcast([P, NB, E])
            m3 = sb.tile([P, NB * E], fp32)
            nc.vector.tensor_tensor(
                out=m3[:].rearrange("p (c s) -> p c s", s=E), in0=q3, in1=bof3,
                op=mybir.AluOpType.is_gt)
            t3 = sb.tile([P, NB * E], fp32)
            nc.vector.tensor_tensor(
                out=t3[:].rearrange("p (c s) -> p c s", s=E),
                in0=m3[:].rearrange("p (c s) -> p c s", s=E), in1=src3,
                op=mybir.AluOpType.mult)
            srcq = sb.tile([P, NB], fp32)
            nc.vector.tensor_reduce(
                out=srcq[:].rearrange("p (c one) -> p c one", one=1),
                in_=t3[:].rearrange("p (c s) -> p c s", s=E),
                axis=mybir.AxisListType.X, op=mybir.AluOpType.max)
            nc.vector.tensor_tensor(
                out=srcq[:], in0=srcq[:], in1=qiot[:],
                op=mybir.AluOpType.add)
            srci = sb.tile([P, NB], dt.int32)
            nc.vector.tensor_copy(srci[:], srcq[:])
            # exact region id from src: s = floor(src / CL)
            sqf = sb.tile([P, NB], fp32)
            nc.vector.tensor_scalar(
                out=sqf[:], in0=srcq[:], scalar1=1.0 / CL, scalar2=None,
                op0=mybir.AluOpType.mult)
            sqi = sb.tile([P, NB], dt.int32)
            nc.vector.tensor_copy(sqi[:], sqf[:])
            nc.vector.tensor_copy(sqf[:], sqi[:])
            m2 = sb.tile([P, NB * E], fp32)
            nc.vector.tensor_tensor(
                out=m2[:].rearrange("p (c s) -> p c s", s=E),
                in0=sqf[:].rearrange("p (c one) -> p c one", one=1)
                .to_broadcast([P, NB, E]),
                in1=siot[:].rearrange("p (one s) -> p one s", one=1)
                .to_broadcast([P, NB, E]),
                op=mybir.AluOpType.is_equal)
            nc.vector.tensor_tensor(
                out=m2[:].rearrange("p (c s) -> p c s", s=E),
                in0=m2[:].rearrange("p (c s) -> p c s", s=E), in1=pos3,
                op=mybir.AluOpType.mult)
            posq = sb.tile([P, NB], fp32)
            nc.vector.tensor_reduce(
                out=posq[:].rearrange("p (c one) -> p c one", one=1),
                in_=m2[:].rearrange("p (c s) -> p c s", s=E),
                axis=mybir.AxisListType.X, op=mybir.AluOpType.max)
            nc.vector.tensor_tensor(
                out=posq[:], in0=posq[:], in1=qiot[:],
                op=mybir.AluOpType.add)

            # ---- compaction gather of (id+1, gate) pairs for the B set
            pairsB = sb.tile([P, NB * 2], fp32)
            nc.vector.memset(pairsB[:], 0.0)
            for c in range(NB):
                nc.gpsimd.indirect_dma_start(
                    out=pairsB[:, c * 2:(c + 1) * 2], out_offset=None,
                    in_=igd_rcv[:], in_offset=bass.IndirectOffsetOnAxis(
                        ap=srci[:, c:c + 1], axis=0),
                    bounds_check=RCV - 1, oob_is_err=False)
            ivB = pairsB[:].rearrange("p (c two) -> p c two", two=2)[:, :, 0:1]
            gvB = pairsB[:].rearrange("p (c two) -> p c two", two=2)[:, :, 1:2]
            vB8 = sb.tile([P, NB], dt.uint8)
            nc.vector.tensor_scalar(
                out=vB8[:], in0=ivB, scalar1=0.5, scalar2=None,
                op0=mybir.AluOpType.is_gt)
            keep8 = sb.tile([P, NB], dt.uint8)
            nc.vector.tensor_scalar(
                out=keep8[:], in0=posq[:], scalar1=float(C) - 0.5, scalar2=None,
                op0=mybir.AluOpType.is_lt)
            nc.vector.tensor_tensor(
                out=vB8[:], in0=vB8[:], in1=keep8[:],
                op=mybir.AluOpType.mult)
            idm1B = sb.tile([P, NB], fp32)
            nc.vector.tensor_scalar_add(
                idm1B[:].rearrange("p (c one) -> p c one", one=1), ivB, -1.0)
            idxfB = sb.tile([P, NB], fp32)
            nc.vector.select(idxfB[:], vB8[:], idm1B[:], bigB[:])
            idxB = sb.tile([P, NB], dt.int32)
            nc.vector.tensor_copy(idxB[:], idxfB[:])
            gateB = sb.tile([P, NB], fp32)
            nc.vector.tensor_copy(
                gateB[:].rearrange("p (c one) -> p c one", one=1), gvB)

            # ---- gather B tokens + transpose into dispT_B
            for c in range(NB):
                gx = sbg.tile([P, M], bf16, tag="gx")
                nc.vector.memset(gx[:], 0.0)
                nc.gpsimd.indirect_dma_start(
                    out=gx[:], out_offset=None, in_=xb[:],
                    in_offset=bass.IndirectOffsetOnAxis(
                        ap=idxB[:, c:c + 1], axis=0),
                    bounds_check=T - 1, oob_is_err=False)
                for mm in range(MC):
                    ptg = psT.tile([P, P], fp32, tag="ptg")
                    nc.tensor.matmul(
                        ptg[:], lhsT=gx[:, mm * P:(mm + 1) * P],
                        rhs=idb[:], start=True, stop=True)
                    nc.vector.tensor_copy(
                        dispT_B[:, mm * NBS + c * P:mm * NBS + (c + 1) * P],
                        ptg[:])

            # ---- A-drop mask (uses off_me, available post-A2A) + A second
            # layer + output scatter
            posA = sb.tile([P, NA], fp32)
            nc.vector.tensor_tensor(
                out=posA[:].rearrange("p (c one) -> p c one", one=1),
                in0=qiot[:, 0:NA].rearrange("p (c one) -> p c one", one=1),
                in1=bct[:, 24:25].rearrange("p (c one) -> p c one", one=1)
                .to_broadcast([P, NA, 1]),
                op=mybir.AluOpType.add)
            keepA = sb.tile([P, NA], dt.uint8)
            nc.vector.tensor_scalar(
                out=keepA[:], in0=posA[:], scalar1=float(C) - 0.5, scalar2=None,
                op0=mybir.AluOpType.is_lt)
            nc.vector.tensor_tensor(
                out=keepA[:], in0=keepA[:], in1=vA8[:],
                op=mybir.AluOpType.mult)
            idxfAo = sb.tile([P, NA], fp32)
            nc.vector.select(idxfAo[:], keepA[:], idm1A[:], bigA[:])
            idxAo = sb.tile([P, NA], dt.int32)
            nc.vector.tensor_copy(idxAo[:], idxfAo[:])

            for s5 in range(NA):
                po0 = ps2.tile([P, 512], fp32, tag="po")
                po1 = ps2.tile([P, 512], fp32, tag="po")
                for d in range(DC):
                    lhs = hT_A[:, d * NAS + s5 * P:d * NAS + (s5 + 1) * P]
                    nc.tensor.matmul(
                        po0[:], lhsT=lhs, rhs=w2t[:, d * M:d * M + 512],
                        start=(d == 0), stop=(d == DC - 1))
                    nc.tensor.matmul(
                        po1[:], lhsT=lhs, rhs=w2t[:, d * M + 512:(d + 1) * M],
                        start=(d == 0), stop=(d == DC - 1))
                st = sbst.tile([P, M], fp32, tag="st")
                for hh, po in ((0, po0), (1, po1)):
                    nc.vector.tensor_tensor(
                        out=st[:, hh * 512:(hh + 1) * 512], in0=po[:],
                        in1=b2t[:, hh * 512:(hh + 1) * 512],
                        op=mybir.AluOpType.add)
                nc.vector.tensor_scalar_mul(
                    st[:], st[:], gateA[:, s5:s5 + 1])
                nc.gpsimd.indirect_dma_start(
                    out=outd[:], out_offset=bass.IndirectOffsetOnAxis(
                        ap=idxAo[:, s5:s5 + 1], axis=0),
                    in_=st[:], in_offset=None,
                    bounds_check=T - 1, oob_is_err=False)

            # ---- B first layer (w1 stream pass 2)
            for d in range(DC):
                w1t = sbw1.tile([P, M], bf16, tag="w1t")
                nc.sync.dma_start(w1t[:], w1p[d])
                pA = psW.tile([P, 512], fp32, tag="pA")
                pB = psW.tile([P, 512], fp32, tag="pB")
                for mc in range(MC):
                    lhs = w1t[:, mc * P:(mc + 1) * P]
                    nc.tensor.matmul(
                        pA[:], lhsT=lhs,
                        rhs=dispT_B[:, mc * NBS:mc * NBS + 512],
                        start=(mc == 0), stop=(mc == MC - 1))
                    nc.tensor.matmul(
                        pB[:], lhsT=lhs,
                        rhs=dispT_B[:, mc * NBS + 512:(mc + 1) * NBS],
                        start=(mc == 0), stop=(mc == MC - 1))
                nc.scalar.activation(
                    hT_B[:, d * NBS:d * NBS + 512], pA[:],
                    mybir.ActivationFunctionType.Relu,
                    bias=b1t[:, d:d + 1], scale=1.0)
                nc.scalar.activation(
                    hT_B[:, d * NBS + 512:(d + 1) * NBS], pB[:],
                    mybir.ActivationFunctionType.Relu,
                    bias=b1t[:, d:d + 1], scale=1.0)

            # ---- B second layer + output scatter
            for s5 in range(NB):
                po0 = ps2.tile([P, 512], fp32, tag="po")
                po1 = ps2.tile([P, 512], fp32, tag="po")
                for d in range(DC):
                    lhs = hT_B[:, d * NBS + s5 * P:d * NBS + (s5 + 1) * P]
                    nc.tensor.matmul(
                        po0[:], lhsT=lhs, rhs=w2t[:, d * M:d * M + 512],
                        start=(d == 0), stop=(d == DC - 1))
                    nc.tensor.matmul(
                        po1[:], lhsT=lhs, rhs=w2t[:, d * M + 512:(d + 1) * M],
                        start=(d == 0), stop=(d == DC - 1))
                st = sbst.tile([P, M], fp32, tag="st")
                for hh, po in ((0, po0), (1, po1)):
                    nc.vector.tensor_tensor(
                        out=st[:, hh * 512:(hh + 1) * 512], in0=po[:],
                        in1=b2t[:, hh * 512:(hh + 1) * 512],
                        op=mybir.AluOpType.add)
                nc.vector.tensor_scalar_mul(
                    st[:], st[:], gateB[:, s5:s5 + 1])
                nc.gpsimd.indirect_dma_start(
                    out=outd[:], out_offset=bass.IndirectOffsetOnAxis(
                        ap=idxB[:, s5:s5 + 1], axis=0),
                    in_=st[:], in_offset=None,
                    bounds_check=T - 1, oob_is_err=False)

    nc.compile()
    return nc


def _make_w64():
    w = np.zeros((LE, LE + E), dtype=np.float32)
    for tip in range(LT):
        for ep in range(E):
            r = tip * E + ep
            for ti in range(LT):
                if tip < ti:
                    w[r, ti * E + ep] = 1.0
            w[r, LE + ep] = 1.0
    return w


def _prep_inputs(x, wg, w1, b1, w2, b2):
    bf = ml_dtypes.bfloat16
    tokens = np.ascontiguousarray(x.reshape(T, M)).astype(np.float32)
    xT = np.ascontiguousarray(tokens.T)
    xb = tokens.astype(bf)
    wgf = np.ascontiguousarray(wg.astype(np.float32))
    eiota = np.tile(np.arange(E, dtype=np.float32), LT)[None, :].repeat(P, 0)
    triu = np.triu(np.ones((P, P), dtype=np.float32))
    identf = np.eye(P, dtype=np.float32)
    identb = np.eye(P).astype(bf)
    w64 = _make_w64()
    ecl = np.tile(np.arange(E, dtype=np.float32) * CL, LT)[None, :].repeat(P, 0)
    qiota = (np.arange(NB, dtype=np.float32)[None, :] * P
             + np.arange(P, dtype=np.float32)[:, None]).copy()
    siota = np.arange(E, dtype=np.float32)[None, :].repeat(P, 0)
    cpak = np.concatenate(
        [eiota, triu, identf, ecl,
         np.zeros((P, 8), np.float32), qiota, siota], axis=1)
    in_maps = []
    for e in range(E):
        w1e = np.ascontiguousarray(w1[e]).astype(bf)
        w1pk = np.ascontiguousarray(
            w1e.reshape(MC, P, DC, P).transpose(2, 1, 0, 3))
        w2e = np.ascontiguousarray(w2[e]).astype(bf)
        w2pk = np.ascontiguousarray(
            w2e.reshape(DC, P, M).transpose(1, 0, 2))
        tokp1 = (e * TSH + np.arange(TSH, dtype=np.float32)
                 .reshape(LT, P).T + 1.0).copy()
        cpk = cpak.copy()
        cpk[:, 384:392] = tokp1
        meloc = (e * CL + 1 + qiota[:, :NA]).astype(np.int32)
        perc = np.zeros((1, 24), dtype=np.float32)
        for s in range(E):
            perc[0, s] = 1.0 if s == e else 0.0
            perc[0, 8 + s] = s * CL + (NAS + 1 if s == e else 1)
            perc[0, 16 + s] = float(NAS) if s == e else 0.0
        in_maps.append({
            "xTs": np.ascontiguousarray(xT[:, e * TSH:(e + 1) * TSH]),
            "xb": xb, "wg": wgf,
            "w1p": w1pk, "w2p": w2pk,
            "b1v": np.ascontiguousarray(b1[e]).astype(np.float32),
            "b2b": np.tile(np.asarray(b2[e], dtype=np.float32), (P, 1)),
            "cpakd": np.ascontiguousarray(cpk),
            "identb": identb, "w64d": w64,
            "melocd": meloc, "percd": perc,
        })
    return in_maps


def kernel(x, wg, w1, b1, w2, b2, _trace=False):
    if "nc" not in _CACHE:
        _CACHE["nc"] = _build_nc()
    nc = _CACHE["nc"]
    in_maps = _prep_inputs(
        np.asarray(x), np.asarray(wg), np.asarray(w1),
        np.asarray(b1), np.asarray(w2), np.asarray(b2))
    res = run_bass_kernel_spmd(nc, in_maps, list(range(E)), trace=_trace)
    _CACHE["last_results"] = res
    full = np.zeros((T, M), dtype=np.float32)
    for e in range(E):
        full += res.results[e]["out"]
    return full.reshape(B, S, M)


# revision 33
# speedup vs baseline: 1.0454x; 1.0246x over previous
"""MoE top-1 routing kernel for 8 TRN2 NeuronCores (expert parallelism).

Self-contained: takes full inputs, shards experts across 8 cores, returns the
full output (host sums the 8 disjoint per-expert partials).

v5 design (local-first pipelining, single collective):
- Gating is token-sharded: each core computes fp32 logits for its own 1024
  tokens, then DVE softmax/argmax and local (shard-internal) queue positions
  via one triangular matmul. No counts-AllGather is needed: each shard
  scatters (token_id+1, gate) into per-(shard,expert) regions of a [E*CL,2]
  send buffer at LOCAL positions (CL=512 rows/region, row 0 = count), and one
  32 KB AllToAll delivers region e to core e.
- Latency hiding: tokens of a core's OWN shard choosing its OWN expert (the
  "A set", local pos < 256) are known before the AllToAll; the core gathers
  them from the replicated token buffer and starts the FFN on them (2 chunks
  of 128 slots) while the collective + receive-side compaction for the
  remaining "B set" (8 chunks) is still in flight. Queue order is irrelevant
  to the math: outputs are scattered back by token id, and capacity drops
  (global pos >= C) are applied exactly at output-scatter time using the
  received per-shard counts.
- Receive-side compaction is a computed gather: per-shard counts (embedded at
  region row 0) -> prefix sums on DVE -> per-slot source index + global
  position -> one level of 8B-row indirect gathers, then the usual 2KB-row
  token gathers from the bf16 token buffer.
- FFN in bf16: w2 resident in SBUF (8 MB), w1 streamed once per pass (A then
  B), fused bias+ReLU on the scalar engine, gate-scaled rows scattered into
  the output by token id.
"""
import numpy as np
import ml_dtypes
from contextlib import ExitStack

import concourse.bass as bass
import concourse.tile as tile
from concourse import bacc, mybir
from concourse.bass_utils import run_bass_kernel_spmd

dt = mybir.dt

B, S, M, E, DFF = 4, 2048, 1024, 8, 4096
T = B * S
C = int(1.25 * T / E)          # 1280 capacity per expert
P = 128
MC = M // P                    # 8
DC = DFF // P                  # 32
TSH = T // E                   # 1024 tokens per shard
LT = TSH // P                  # 8
LE = LT * E                    # 64
CL = 512                       # rows per (shard, expert) region (row0=count)
RCV = E * CL                   # 4096
NA = 2                         # local-first chunks (256 slots)
NAS = NA * P
NB = 8                         # remote/compacted chunks (1024 slots)
NBS = NB * P
BIG = 1.5e9

_CACHE = {}


def _build_nc():
    nc = bacc.Bacc("TRN2", target_bir_lowering=False, debug=False)

    xTs = nc.dram_tensor("xTs", [M, TSH], dt.float32, kind="ExternalInput")
    xb = nc.dram_tensor("xb", [T, M], dt.bfloat16, kind="ExternalInput")
    wg = nc.dram_tensor("wg", [M, E], dt.float32, kind="ExternalInput")
    w1p = nc.dram_tensor("w1p", [DC, P, MC, P], dt.bfloat16, kind="ExternalInput")
    w2p = nc.dram_tensor("w2p", [P, DC, M], dt.bfloat16, kind="ExternalInput")
    b1v = nc.dram_tensor("b1v", [DFF], dt.float32, kind="ExternalInput")
    b2b = nc.dram_tensor("b2b", [P, M], dt.float32, kind="ExternalInput")
    cpakd = nc.dram_tensor("cpakd", [P, 408], dt.float32, kind="ExternalInput")
    identb = nc.dram_tensor("identb", [P, P], dt.bfloat16, kind="ExternalInput")
    w64d = nc.dram_tensor("w64d", [LE, LE + E], dt.float32, kind="ExternalInput")
    melocd = nc.dram_tensor("melocd", [P, NA], dt.int32, kind="ExternalInput")
    percd = nc.dram_tensor("percd", [1, 24], dt.float32, kind="ExternalInput")
    outd = nc.dram_tensor("out", [T, M], dt.float32, kind="ExternalOutput")

    igd_loc = nc.dram_tensor("igd_loc", [RCV, 2], dt.float32)
    igd_ps = [nc.dram_tensor(f"igd_p{i}", [RCV, 2], dt.float32)
              for i in range(4)]
    igd_rcv = nc.dram_tensor("igd_rcv", [RCV, 2], dt.float32)
    scrd = nc.dram_tensor("scrd", [1, 32], dt.float32)
    wrm_l = nc.dram_tensor("wrm_l", [8, 2], dt.float32)
    wrm_a = nc.dram_tensor("wrm_a", [64, 2], dt.float32, addr_space="Shared")

    fp32 = dt.float32
    bf16 = dt.bfloat16

    with tile.TileContext(nc) as tc, ExitStack() as ctx:
        sb = ctx.enter_context(tc.tile_pool(name="sb", bufs=1))
        sbx = ctx.enter_context(tc.tile_pool(name="sbx", bufs=6))
        sbw1 = ctx.enter_context(tc.tile_pool(name="sbw1", bufs=4))
        sbg = ctx.enter_context(tc.tile_pool(name="sbg", bufs=4))
        sbst = ctx.enter_context(tc.tile_pool(name="sbst", bufs=2))

        # ---- warmup collective: absorbs CC channel init + start barrier
        wz = sb.tile([8, 2], fp32)
        nc.vector.memset(wz[:], 0.0)
        nc.sync.dma_start(wrm_l[:], wz[:])
        nc.gpsimd.collective_compute(
            "AllGather", mybir.AluOpType.bypass,
            ins=[wrm_l[:]], outs=[wrm_a[:]],
            replica_groups=[list(range(E))])

        # ---- const loads (sync queue; xTs chunks follow in the gating loop)
        wgt = sb.tile([P, MC * E], fp32)
        nc.sync.dma_start(wgt[:], wg[:].rearrange("(mc p) e -> p mc e", p=P))
        cpak = sb.tile([P, 408], fp32)
        nc.sync.dma_start(cpak[:], cpakd[:])
        eit = cpak[:, 0:64]
        trit = cpak[:, 64:192]
        idf = cpak[:, 192:320]
        eclt = cpak[:, 320:384]
        tokp1 = cpak[:, 384:392]
        qiot = cpak[:, 392:400]
        siot = cpak[:, 400:408]
        idb = sb.tile([P, P], bf16)
        nc.sync.dma_start(idb[:], identb[:])
        w64t = sb.tile([LE, LE + E], fp32)
        nc.sync.dma_start(w64t[:], w64d[:])
        meloct = sb.tile([P, NA], dt.int32)
        nc.sync.dma_start(meloct[:], melocd[:])
        percs = sb.tile([1, 24], fp32)
        nc.sync.dma_start(percs[:], percd[:])
        b1t = sb.tile([P, DC], fp32)
        nc.sync.dma_start(b1t[:], b1v[:].rearrange("(d p) -> p d", p=P))
        # prefetch the first w1 chunks before gating so the A first-layer
        # stream is not scheduled behind the routing vector chain
        w1pre = []
        for dd in range(4):
            w1t = sbw1.tile([P, M], bf16, tag="w1t", name=f"w1pre{dd}",
                            uniquify=True)
            nc.sync.dma_start(w1t[:], w1p[dd])
            w1pre.append(w1t)
        # PE warm-up: keep the HAM activity window busy before the logits MMs
        wrmj = sb.tile([P, P], bf16)
        nc.vector.memset(wrmj[:], 0.0)
        # scalar queue: b2 broadcast + zero-prefill of the send buffer
        b2t = sb.tile([P, M], fp32)
        nc.scalar.dma_start(b2t[:], b2b[:])
        zpre = sb.tile([P, RCV * 2 // P], fp32)
        nc.vector.memset(zpre[:], 0.0)
        for i in range(4):
            nc.scalar.dma_start(
                igd_ps[i][:].rearrange("(p c) two -> p c two", p=P), zpre[:])

        ones1 = sb.tile([1, P], fp32)
        nc.vector.memset(ones1[:], 1.0)
        onescol = sb.tile([P, 1], fp32)
        nc.vector.memset(onescol[:], 1.0)
        nines = sb.tile([P, LE], fp32)
        nc.vector.memset(nines[:], 9.0)
        huget = sb.tile([P, LE], fp32)
        nc.vector.memset(huget[:], BIG)
        bigA = sb.tile([P, NA], fp32)
        nc.vector.memset(bigA[:], BIG)
        bigB = sb.tile([P, NB], fp32)
        nc.vector.memset(bigB[:], BIG)

        # ---- gating: fp32 logits for my 1024 tokens
        eg_stk = sb.tile([P, LT * 2], fp32)
        lg_stk = sb.tile([P, LE], fp32)
        lgT = sb.tile([8, TSH], fp32)
        with tc.tile_pool(name="psg", bufs=2, space="PSUM") as psg:
            pwrm = psg.tile([P, P], fp32, tag="pwrm")
            for _w in range(24):
                nc.tensor.matmul(pwrm[:], lhsT=wrmj[:], rhs=idb[:],
                                 start=(_w == 0), stop=(_w == 23))
            pl0 = psg.tile([8, 512], fp32, tag="pl0")
            pl1 = psg.tile([8, 512], fp32, tag="pl1")
            for k in range(MC):
                xt0 = sbx.tile([P, 512], fp32, tag="xt")
                nc.sync.dma_start(xt0[:], xTs[k * P:(k + 1) * P, 0:512])
                nc.tensor.matmul(
                    pl0[:], lhsT=wgt[:, k * E:(k + 1) * E], rhs=xt0[:],
                    start=(k == 0), stop=(k == MC - 1))
                xt1 = sbx.tile([P, 512], fp32, tag="xt")
                nc.sync.dma_start(xt1[:], xTs[k * P:(k + 1) * P, 512:1024])
                nc.tensor.matmul(
                    pl1[:], lhsT=wgt[:, k * E:(k + 1) * E], rhs=xt1[:],
                    start=(k == 0), stop=(k == MC - 1))
            nc.vector.tensor_copy(lgT[:, 0:512], pl0[:])
            nc.vector.tensor_copy(lgT[:, 512:1024], pl1[:])
            for ti in range(LT):
                pq = psg.tile([P, E], fp32, tag="pq")
                nc.tensor.transpose(
                    out=pq[:], in_=lgT[:, ti * P:(ti + 1) * P],
                    identity=idf[:8, :8])
                nc.vector.tensor_copy(lg_stk[:, ti * E:(ti + 1) * E], pq[:])
        lg3 = lg_stk[:].rearrange("p (ti e) -> p ti e", e=E)
        mx_stk = sb.tile([P, LT], fp32)
        nc.vector.tensor_reduce(
            out=mx_stk[:], in_=lg3, axis=mybir.AxisListType.X,
            op=mybir.AluOpType.max)
        mxb = mx_stk[:].rearrange("p (ti one) -> p ti one", one=1).to_broadcast([P, LT, E])
        ls = sb.tile([P, LE], fp32)
        nc.vector.tensor_tensor(
            out=ls[:].rearrange("p (ti e) -> p ti e", e=E), in0=lg3, in1=mxb,
            op=mybir.AluOpType.subtract)
        ex = sb.tile([P, LE], fp32)
        nc.scalar.activation(ex[:], ls[:], mybir.ActivationFunctionType.Exp)
        s_stk = sb.tile([P, LT], fp32)
        nc.vector.tensor_reduce(
            out=s_stk[:], in_=ex[:].rearrange("p (ti e) -> p ti e", e=E),
            axis=mybir.AxisListType.X, op=mybir.AluOpType.add)
        nc.vector.reciprocal(
            eg_stk[:].rearrange("p (ti two) -> p ti two", two=2)[:, :, 1:2],
            s_stk[:].rearrange("p (ti one) -> p ti one", one=1))
        oh = sb.tile([P, LE], dt.uint8)
        nc.vector.tensor_tensor(
            out=oh[:].rearrange("p (ti e) -> p ti e", e=E), in0=lg3, in1=mxb,
            op=mybir.AluOpType.is_equal)
        msk = sb.tile([P, LE], fp32)
        nc.vector.select(msk[:], oh[:], eit[:], nines[:])
        nc.vector.tensor_reduce(
            out=eg_stk[:].rearrange("p (ti two) -> p ti two", two=2)[:, :, 0:1],
            in_=msk[:].rearrange("p (ti e) -> p ti e", e=E),
            axis=mybir.AxisListType.X, op=mybir.AluOpType.min)
        eidx_v = eg_stk[:].rearrange("p (ti two) -> p ti two", two=2)[:, :, 0:1]
        gate_v = eg_stk[:].rearrange("p (ti two) -> p ti two", two=2)[:, :, 1:2]
        mine_all = sb.tile([P, LE], fp32)
        nc.vector.tensor_tensor(
            out=mine_all[:].rearrange("p (ti e) -> p ti e", e=E),
            in0=eidx_v.to_broadcast([P, LT, E]),
            in1=eit[:].rearrange("p (ti e) -> p ti e", e=E),
            op=mybir.AluOpType.is_equal)

        # ---- local queue positions + per-expert counts (all shard-local)
        offsb = sb.tile([1, LE + E], fp32)
        palls = sb.tile([P, LE], fp32)
        with tc.tile_pool(name="ppb", bufs=1, space="PSUM") as ppb:
            pts = ppb.tile([LE, 1], fp32, tag="pts")
            nc.tensor.matmul(pts[:], lhsT=mine_all[:], rhs=onescol[:],
                             start=True, stop=True)
            tscol = sb.tile([LE, 1], fp32)
            nc.vector.tensor_copy(tscol[:], pts[:])
            poffs = ppb.tile([1, LE + E], fp32, tag="poffs")
            nc.tensor.matmul(poffs[:], lhsT=tscol[:], rhs=w64t[:],
                             start=True, stop=True)
            nc.vector.tensor_copy(offsb[:], poffs[:])
            # counts c_{me,e} -> row 0 of each region of the send buffer
            nc.sync.dma_start(
                igd_ps[0][:].rearrange("(e cl) two -> cl e two", cl=CL)
                [0:1, :, 0:1],
                offsb[:, LE:LE + E].rearrange("p (e one) -> p e one", one=1))
            pall = ppb.tile([P, LE], fp32, tag="pall")
            nc.tensor.matmul(pall[:], lhsT=trit[:], rhs=mine_all[:],
                             start=True, stop=False)
            nc.tensor.matmul(pall[:], lhsT=ones1[:], rhs=offsb[:, 0:LE],
                             start=False, stop=True)
            nc.vector.tensor_copy(palls[:], pall[:])
        mu8 = sb.tile([P, LE], dt.uint8)
        nc.vector.tensor_scalar(
            out=mu8[:], in0=mine_all[:], scalar1=0.5, scalar2=None,
            op0=mybir.AluOpType.is_gt)
        cu8 = sb.tile([P, LE], dt.uint8)
        nc.vector.tensor_scalar(
            out=cu8[:], in0=palls[:], scalar1=float(CL) - 0.5, scalar2=None,
            op0=mybir.AluOpType.is_lt)
        au8 = sb.tile([P, LE], dt.uint8)
        nc.vector.tensor_tensor(
            out=au8[:], in0=mu8[:], in1=cu8[:], op=mybir.AluOpType.mult)
        s1 = sb.tile([P, LE], fp32)
        nc.vector.select(s1[:], au8[:], palls[:], huget[:])
        dstf = sb.tile([P, LE], fp32)
        nc.vector.tensor_tensor(
            out=dstf[:], in0=s1[:], in1=eclt[:], op=mybir.AluOpType.add)
        rowmin = sb.tile([P, LT], fp32)
        nc.vector.tensor_reduce(
            out=rowmin[:].rearrange("p (ti one) -> p ti one", one=1),
            in_=dstf[:].rearrange("p (ti e) -> p ti e", e=E),
            axis=mybir.AxisListType.X, op=mybir.AluOpType.min)
        dsti = sb.tile([P, LT], dt.int32)
        nc.vector.tensor_copy(dsti[:], rowmin[:])
        pairs = sb.tile([P, LT * 2], fp32)
        nc.vector.tensor_copy(
            pairs[:].rearrange("p (t two) -> p t two", two=2)[:, :, 0:1],
            tokp1[:].rearrange("p (t one) -> p t one", one=1))
        nc.vector.tensor_copy(
            pairs[:].rearrange("p (t two) -> p t two", two=2)[:, :, 1:2],
            gate_v)

        # ---- scatter (id+1, gate) into per-(shard,expert) regions.
        # Four destination tensors -> four independent 2-link WAW chains
        # instead of one 8-link chain; merged below on the scalar queue
        # (kept off the sync queue, whose w1-stream buffer waits depend
        # transitively on this merge).
        for t in range(LT):
            nc.gpsimd.indirect_dma_start(
                out=igd_ps[t % 4][:], out_offset=bass.IndirectOffsetOnAxis(
                    ap=dsti[:, t:t + 1], axis=0),
                in_=pairs[:, 2 * t:2 * t + 2], in_offset=None,
                bounds_check=RCV - 1, oob_is_err=False)
        mrg = sb.tile([P, RCV * 2 // P], fp32)
        mrgb = sb.tile([P, RCV * 2 // P], fp32)
        mrgc = sb.tile([P, RCV * 2 // P], fp32)
        mrgd = sb.tile([P, RCV * 2 // P], fp32)
        for i, dst in enumerate((mrg, mrgb, mrgc, mrgd)):
            nc.scalar.dma_start(
                dst[:], igd_ps[i][:].rearrange("(p c) two -> p c two", p=P))
        nc.vector.tensor_tensor(
            out=mrg[:], in0=mrg[:], in1=mrgb[:], op=mybir.AluOpType.add)
        nc.vector.tensor_tensor(
            out=mrgc[:], in0=mrgc[:], in1=mrgd[:], op=mybir.AluOpType.add)
        nc.vector.tensor_tensor(
            out=mrg[:], in0=mrg[:], in1=mrgc[:], op=mybir.AluOpType.add)
        nc.scalar.dma_start(
            igd_loc[:].rearrange("(p c) two -> p c two", p=P), mrg[:])

        # ---- the single AllToAll: region e -> core e
        # ---- A set: my own tokens for my expert (local pos < NAS), read
        # straight from the four partial send buffers (each slot is nonzero
        # in exactly one) so the FFN can start without waiting for the
        # merge or the AllToAll.
        pps = []
        for i in range(4):
            pp = sb.tile([P, NA * 2], fp32, name=f"ppA{i}", uniquify=True)
            nc.vector.memset(pp[:], 0.0)
            for c in range(NA):
                nc.gpsimd.indirect_dma_start(
                    out=pp[:, c * 2:(c + 1) * 2], out_offset=None,
                    in_=igd_ps[i][:], in_offset=bass.IndirectOffsetOnAxis(
                        ap=meloct[:, c:c + 1], axis=0),
                    bounds_check=RCV - 1, oob_is_err=False)
            pps.append(pp)
        pairsA = sb.tile([P, NA * 2], fp32)
        nc.vector.tensor_tensor(
            out=pairsA[:], in0=pps[0][:], in1=pps[1][:],
            op=mybir.AluOpType.add)
        nc.vector.tensor_tensor(
            out=pps[2][:], in0=pps[2][:], in1=pps[3][:],
            op=mybir.AluOpType.add)
        nc.vector.tensor_tensor(
            out=pairsA[:], in0=pairsA[:], in1=pps[2][:],
            op=mybir.AluOpType.add)

        nc.gpsimd.collective_compute(
            "AllToAll", mybir.AluOpType.bypass,
            ins=[igd_loc[:]], outs=[igd_rcv[:]],
            replica_groups=[list(range(E))])
        ivA = pairsA[:].rearrange("p (c two) -> p c two", two=2)[:, :, 0:1]
        gvA = pairsA[:].rearrange("p (c two) -> p c two", two=2)[:, :, 1:2]
        vA8 = sb.tile([P, NA], dt.uint8)
        nc.vector.tensor_scalar(
            out=vA8[:], in0=ivA, scalar1=0.5, scalar2=None,
            op0=mybir.AluOpType.is_gt)
        idm1A = sb.tile([P, NA], fp32)
        nc.vector.tensor_scalar_add(
            idm1A[:].rearrange("p (c one) -> p c one", one=1), ivA, -1.0)
        idxfA = sb.tile([P, NA], fp32)
        nc.vector.select(idxfA[:], vA8[:], idm1A[:], bigA[:])
        idxAin = sb.tile([P, NA], dt.int32)
        nc.vector.tensor_copy(idxAin[:], idxfA[:])
        gateA = sb.tile([P, NA], fp32)
        nc.vector.tensor_copy(
            gateA[:].rearrange("p (c one) -> p c one", one=1), gvA)

        w2t = sb.tile([P, DC * M], bf16)
        hT_B = sb.tile([P, DC * NBS], bf16)
        # A's hidden activations alias the first DC*NAS columns of hT_B:
        # A-w2's reads complete exactly when B-w1's writes begin.
        hT_A = hT_B
        dispT_A = sb.tile([P, MC * NAS], bf16)
        dispT_B = sb.tile([P, MC * NBS], bf16)

        with (
            tc.tile_pool(name="psT", bufs=2, space="PSUM") as psT,
            tc.tile_pool(name="psW", bufs=2, space="PSUM") as psW,
            tc.tile_pool(name="ps2", bufs=2, space="PSUM") as ps2,
        ):
            # ---- gather A tokens + transpose into dispT_A
            for c in range(NA):
                gx = sbg.tile([P, M], bf16, tag="gx")
                nc.vector.memset(gx[:], 0.0)
                nc.gpsimd.indirect_dma_start(
                    out=gx[:], out_offset=None, in_=xb[:],
                    in_offset=bass.IndirectOffsetOnAxis(
                        ap=idxAin[:, c:c + 1], axis=0),
                    bounds_check=T - 1, oob_is_err=False)
                for mm in range(MC):
                    ptg = psT.tile([P, P], fp32, tag="ptg")
                    nc.tensor.matmul(
                        ptg[:], lhsT=gx[:, mm * P:(mm + 1) * P],
                        rhs=idb[:], start=True, stop=True)
                    nc.vector.tensor_copy(
                        dispT_A[:, mm * NAS + c * P:mm * NAS + (c + 1) * P],
                        ptg[:])
            # ---- A first layer (w1 stream pass 1)
            for d in range(DC):
                if d < 4:
                    w1t = w1pre[d]
                else:
                    w1t = sbw1.tile([P, M], bf16, tag="w1t")
                    nc.sync.dma_start(w1t[:], w1p[d])
                pA = psW.tile([P, NAS], fp32, tag="pA")
                for mc in range(MC):
                    nc.tensor.matmul(
                        pA[:], lhsT=w1t[:, mc * P:(mc + 1) * P],
                        rhs=dispT_A[:, mc * NAS:(mc + 1) * NAS],
                        start=(mc == 0), stop=(mc == MC - 1))
                nc.scalar.activation(
                    hT_A[:, d * NAS:(d + 1) * NAS], pA[:],
                    mybir.ActivationFunctionType.Relu,
                    bias=b1t[:, d:d + 1], scale=1.0)

            # ---- w2 resident load (sync queue: after the A w1 stream)
            for q in range(4):
                nc.sync.dma_start(
                    w2t[:, q * 8 * M:(q + 1) * 8 * M],
                    w2p[:, q * 8:(q + 1) * 8, :])

            # ---- B-prep: counts -> prefix sums -> per-slot src index + pos
            cnt8 = sb.tile([1, E], fp32)
            nc.gpsimd.dma_start(
                cnt8[:].rearrange("p (a s) -> p a s", a=1),
                igd_rcv[:].rearrange("(s cl) two -> two cl s", cl=CL)
                [0:1, 0:1, :])
            mem8 = percs[:, 0:8]
            bvec = percs[:, 8:16]
            r0m1 = percs[:, 16:24]
            cprime = sb.tile([1, E], fp32)
            nc.vector.tensor_scalar(
                out=cprime[:], in0=mem8, scalar1=float(-NAS), scalar2=None,
                op0=mybir.AluOpType.mult)
            nc.vector.tensor_tensor(
                out=cprime[:], in0=cnt8[:], in1=cprime[:],
                op=mybir.AluOpType.add)
            nc.vector.tensor_scalar(
                out=cprime[:], in0=cprime[:], scalar1=0.0, scalar2=None,
                op0=mybir.AluOpType.max)

            def _incl_prefix(src):
                a1 = sb.tile([1, E], fp32)
                nc.vector.tensor_copy(a1[:], src[:])
                nc.vector.tensor_tensor(
                    out=a1[:, 1:8], in0=src[:, 1:8], in1=src[:, 0:7],
                    op=mybir.AluOpType.add)
                a2 = sb.tile([1, E], fp32)
                nc.vector.tensor_copy(a2[:], a1[:])
                nc.vector.tensor_tensor(
                    out=a2[:, 2:8], in0=a1[:, 2:8], in1=a1[:, 0:6],
                    op=mybir.AluOpType.add)
                a3 = sb.tile([1, E], fp32)
                nc.vector.tensor_copy(a3[:], a2[:])
                nc.vector.tensor_tensor(
                    out=a3[:, 4:8], in0=a2[:, 4:8], in1=a2[:, 0:4],
                    op=mybir.AluOpType.add)
                ex_ = sb.tile([1, E], fp32)
                nc.vector.memset(ex_[:], 0.0)
                nc.vector.tensor_copy(ex_[:, 1:8], a3[:, 0:7])
                return ex_

            offx = _incl_prefix(cnt8)     # exclusive prefix of full counts
            boff = _incl_prefix(cprime)   # exclusive prefix of B counts
            boffm = sb.tile([1, E], fp32)
            nc.vector.tensor_scalar_add(boffm[:], boff[:], -0.5)
            srcv = sb.tile([1, E], fp32)
            nc.vector.tensor_tensor(
                out=srcv[:], in0=bvec, in1=boff[:],
                op=mybir.AluOpType.subtract)
            posoffv = sb.tile([1, E], fp32)
            nc.vector.tensor_tensor(
                out=posoffv[:], in0=offx[:], in1=r0m1,
                op=mybir.AluOpType.add)
            nc.vector.tensor_tensor(
                out=posoffv[:], in0=posoffv[:], in1=boff[:],
                op=mybir.AluOpType.subtract)
            offme1 = sb.tile([1, E], fp32)
            nc.vector.tensor_tensor(
                out=offme1[:], in0=mem8, in1=offx[:],
                op=mybir.AluOpType.mult)
            scrt = sb.tile([1, 32], fp32)
            nc.vector.tensor_copy(scrt[:, 0:8], boffm[:])
            nc.vector.tensor_copy(scrt[:, 8:16], srcv[:])
            nc.vector.tensor_copy(scrt[:, 16:24], posoffv[:])
            nc.vector.tensor_reduce(
                out=scrt[:, 24:25].rearrange("p (a s) -> p a s", a=1),
                in_=offme1[:].rearrange("p (a s) -> p a s", a=1),
                axis=mybir.AxisListType.X, op=mybir.AluOpType.add)
            nc.gpsimd.dma_start(scrd[:], scrt[:])
            bct = sb.tile([P, 32], fp32)
            nc.gpsimd.dma_start(bct[:], scrd[:].to_broadcast([P, 32]))

            q3 = qiot[:].rearrange("p (c one) -> p c one", one=1) \
                .to_broadcast([P, NB, E])
            bof3 = bct[:, 0:8].rearrange("p (one s) -> p one s", one=1) \
                .to_broadcast([P, NB, E])
            src3 = bct[:, 8:16].rearrange("p (one s) -> p one s", one=1) \
                .to_broadcast([P, NB, E])
            pos3 = bct[:, 16:24].rearrange("p (one s) -> p one s", one=1) \
                .to_broadcast([P, NB, E])
            m3 = sb.tile([P, NB * E], fp32)
            nc.vector.tensor_tensor(
                out=m3[:].rearrange("p (c s) -> p c s", s=E), in0=q3, in1=bof3,
                op=mybir.AluOpType.is_gt)
            t3 = sb.tile([P, NB * E], fp32)
            nc.vector.tensor_tensor(
                out=t3[:].rearrange("p (c s) -> p c s", s=E),
                in0=m3[:].rearrange("p (c s) -> p c s", s=E), in1=src3,
                op=mybir.AluOpType.mult)
            srcq = sb.tile([P, NB], fp32)
            nc.vector.tensor_reduce(
                out=srcq[:].rearrange("p (c one) -> p c one", one=1),
                in_=t3[:].rearrange("p (c s) -> p c s", s=E),
                axis=mybir.AxisListType.X, op=mybir.AluOpType.max)
            nc.vector.tensor_tensor(
                out=srcq[:], in0=srcq[:], in1=qiot[:],
                op=mybir.AluOpType.add)
            srci = sb.tile([P, NB], dt.int32)
            nc.vector.tensor_copy(srci[:], srcq[:])
            # exact region id from src: s = floor(src / CL)
            sqf = sb.tile([P, NB], fp32)
            nc.vector.tensor_scalar(
                out=sqf[:], in0=srcq[:], scalar1=1.0 / CL, scalar2=None,
                op0=mybir.AluOpType.mult)
            sqi = sb.tile([P, NB], dt.int32)
            nc.vector.tensor_copy(sqi[:], sqf[:])
            nc.vector.tensor_copy(sqf[:], sqi[:])
            m2 = sb.tile([P, NB * E], fp32)
            nc.vector.tensor_tensor(
                out=m2[:].rearrange("p (c s) -> p c s", s=E),
                in0=sqf[:].rearrange("p (c one) -> p c one", one=1)
                .to_broadcast([P, NB, E]),
                in1=siot[:].rearrange("p (one s) -> p one s", one=1)
                .to_broadcast([P, NB, E]),
                op=mybir.AluOpType.is_equal)
            nc.vector.tensor_tensor(
                out=m2[:].rearrange("p (c s) -> p c s", s=E),
                in0=m2[:].rearrange("p (c s) -> p c s", s=E), in1=pos3,
                op=mybir.AluOpType.mult)
            posq = sb.tile([P, NB], fp32)
            nc.vector.tensor_reduce(
                out=posq[:].rearrange("p (c one) -> p c one", one=1),
                in_=m2[:].rearrange("p (c s) -> p c s", s=E),
                axis=mybir.AxisListType.X, op=mybir.AluOpType.max)
            nc.vector.tensor_tensor(
                out=posq[:], in0=posq[:], in1=qiot[:],
                op=mybir.AluOpType.add)

            # ---- compaction gather of (id+1, gate) pairs for the B set
            pairsB = sb.tile([P, NB * 2], fp32)
            nc.vector.memset(pairsB[:], 0.0)
            for c in range(NB):
                nc.gpsimd.indirect_dma_start(
                    out=pairsB[:, c * 2:(c + 1) * 2], out_offset=None,
                    in_=igd_rcv[:], in_offset=bass.IndirectOffsetOnAxis(
                        ap=srci[:, c:c + 1], axis=0),
                    bounds_check=RCV - 1, oob_is_err=False)
            ivB = pairsB[:].rearrange("p (c two) -> p c two", two=2)[:, :, 0:1]
            gvB = pairsB[:].rearrange("p (c two) -> p c two", two=2)[:, :, 1:2]
            vB8 = sb.tile([P, NB], dt.uint8)
            nc.vector.tensor_scalar(
                out=vB8[:], in0=ivB, scalar1=0.5, scalar2=None,
                op0=mybir.AluOpType.is_gt)
            keep8 = sb.tile([P, NB], dt.uint8)
            nc.vector.tensor_scalar(
                out=keep8[:], in0=posq[:], scalar1=float(C) - 0.5, scalar2=None,
                op0=mybir.AluOpType.is_lt)
            nc.vector.tensor_tensor(
                out=vB8[:], in0=vB8[:], in1=keep8[:],
                op=mybir.AluOpType.mult)
            idm1B = sb.tile([P, NB], fp32)
            nc.vector.tensor_scalar_add(
                idm1B[:].rearrange("p (c one) -> p c one", one=1), ivB, -1.0)
            idxfB = sb.tile([P, NB], fp32)
            nc.vector.select(idxfB[:], vB8[:], idm1B[:], bigB[:])
            idxB = sb.tile([P, NB], dt.int32)
            nc.vector.tensor_copy(idxB[:], idxfB[:])
            gateB = sb.tile([P, NB], fp32)
            nc.vector.tensor_copy(
                gateB[:].rearrange("p (c one) -> p c one", one=1), gvB)

            # ---- A-drop mask (uses off_me, available post-A2A) + A second
            # layer + output scatter
            posA = sb.tile([P, NA], fp32)
            nc.vector.tensor_tensor(
                out=posA[:].rearrange("p (c one) -> p c one", one=1),
                in0=qiot[:, 0:NA].rearrange("p (c one) -> p c one", one=1),
                in1=bct[:, 24:25].rearrange("p (c one) -> p c one", one=1)
                .to_broadcast([P, NA, 1]),
                op=mybir.AluOpType.add)
            keepA = sb.tile([P, NA], dt.uint8)
            nc.vector.tensor_scalar(
                out=keepA[:], in0=posA[:], scalar1=float(C) - 0.5, scalar2=None,
                op0=mybir.AluOpType.is_lt)
            nc.vector.tensor_tensor(
                out=keepA[:], in0=keepA[:], in1=vA8[:],
                op=mybir.AluOpType.mult)
            idxfAo = sb.tile([P, NA], fp32)
            nc.vector.select(idxfAo[:], keepA[:], idm1A[:], bigA[:])
            idxAo = sb.tile([P, NA], dt.int32)
            nc.vector.tensor_copy(idxAo[:], idxfAo[:])

            stA_list = []
            for s5 in range(NA):
                po0 = ps2.tile([P, 512], fp32, tag="po")
                po1 = ps2.tile([P, 512], fp32, tag="po")
                for d in range(DC):
                    lhs = hT_A[:, d * NAS + s5 * P:d * NAS + (s5 + 1) * P]
                    nc.tensor.matmul(
                        po0[:], lhsT=lhs, rhs=w2t[:, d * M:d * M + 512],
                        start=(d == 0), stop=(d == DC - 1))
                    nc.tensor.matmul(
                        po1[:], lhsT=lhs, rhs=w2t[:, d * M + 512:(d + 1) * M],
                        start=(d == 0), stop=(d == DC - 1))
                st = sbst.tile([P, M], fp32, tag="st")
                for hh, po in ((0, po0), (1, po1)):
                    nc.vector.tensor_tensor(
                        out=st[:, hh * 512:(hh + 1) * 512], in0=po[:],
                        in1=b2t[:, hh * 512:(hh + 1) * 512],
                        op=mybir.AluOpType.add)
                nc.vector.tensor_scalar_mul(
                    st[:], st[:], gateA[:, s5:s5 + 1])
                stA_list.append(st)

            # ---- gather B tokens + transpose into dispT_B
            for c in range(NB):
                gx = sbg.tile([P, M], bf16, tag="gx")
                nc.vector.memset(gx[:], 0.0)
                nc.gpsimd.indirect_dma_start(
                    out=gx[:], out_offset=None, in_=xb[:],
                    in_offset=bass.IndirectOffsetOnAxis(
                        ap=idxB[:, c:c + 1], axis=0),
                    bounds_check=T - 1, oob_is_err=False)
                for mm in range(MC):
                    ptg = psT.tile([P, P], fp32, tag="ptg")
                    nc.tensor.matmul(
                        ptg[:], lhsT=gx[:, mm * P:(mm + 1) * P],
                        rhs=idb[:], start=True, stop=True)
                    nc.vector.tensor_copy(
                        dispT_B[:, mm * NBS + c * P:mm * NBS + (c + 1) * P],
                        ptg[:])

            # deferred A output scatters (after the gxB gathers so they
            # do not block the gpsimd queue)
            for s5 in range(NA):
                nc.gpsimd.indirect_dma_start(
                    out=outd[:], out_offset=bass.IndirectOffsetOnAxis(
                        ap=idxAo[:, s5:s5 + 1], axis=0),
                    in_=stA_list[s5][:], in_offset=None,
                    bounds_check=T - 1, oob_is_err=False)

            # ---- B first layer (w1 stream pass 2)
            for d in range(DC):
                w1t = sbw1.tile([P, M], bf16, tag="w1t")
                nc.sync.dma_start(w1t[:], w1p[d])
                pA = psW.tile([P, 512], fp32, tag="pA")
                pB = psW.tile([P, 512], fp32, tag="pB")
                for mc in range(MC):
                    lhs = w1t[:, mc * P:(mc + 1) * P]
                    nc.tensor.matmul(
                        pA[:], lhsT=lhs,
                        rhs=dispT_B[:, mc * NBS:mc * NBS + 512],
                        start=(mc == 0), stop=(mc == MC - 1))
                    nc.tensor.matmul(
                        pB[:], lhsT=lhs,
                        rhs=dispT_B[:, mc * NBS + 512:(mc + 1) * NBS],
                        start=(mc == 0), stop=(mc == MC - 1))
                nc.scalar.activation(
                    hT_B[:, d * NBS:d * NBS + 512], pA[:],
                    mybir.ActivationFunctionType.Relu,
                    bias=b1t[:, d:d + 1], scale=1.0)
                nc.scalar.activation(
                    hT_B[:, d * NBS + 512:(d + 1) * NBS], pB[:],
                    mybir.ActivationFunctionType.Relu,
                    bias=b1t[:, d:d + 1], scale=1.0)

            # ---- B second layer + output scatter
            for s5 in range(NB):
                po0 = ps2.tile([P, 512], fp32, tag="po")
                po1 = ps2.tile([P, 512], fp32, tag="po")
                for d in range(DC):
                    lhs = hT_B[:, d * NBS + s5 * P:d * NBS + (s5 + 1) * P]
                    nc.tensor.matmul(
                        po0[:], lhsT=lhs, rhs=w2t[:, d * M:d * M + 512],
                        start=(d == 0), stop=(d == DC - 1))
                    nc.tensor.matmul(
                        po1[:], lhsT=lhs, rhs=w2t[:, d * M + 512:(d + 1) * M],
                        start=(d == 0), stop=(d == DC - 1))
                st = sbst.tile([P, M], fp32, tag="st")
                for hh, po in ((0, po0), (1, po1)):
                    nc.vector.tensor_tensor(
                        out=st[:, hh * 512:(hh + 1) * 512], in0=po[:],
                        in1=b2t[:, hh * 512:(hh + 1) * 512],
                        op=mybir.AluOpType.add)
                nc.vector.tensor_scalar_mul(
                    st[:], st[:], gateB[:, s5:s5 + 1])
                nc.gpsimd.indirect_dma_start(
                    out=outd[:], out_offset=bass.IndirectOffsetOnAxis(
                        ap=idxB[:, s5:s5 + 1], axis=0),
                    in_=st[:], in_offset=None,
                    bounds_check=T - 1, oob_is_err=False)

    nc.compile()
    return nc


def _make_w64():
    w = np.zeros((LE, LE + E), dtype=np.float32)
    for tip in range(LT):
        for ep in range(E):
            r = tip * E + ep
            for ti in range(LT):
                if tip < ti:
                    w[r, ti * E + ep] = 1.0
            w[r, LE + ep] = 1.0
    return w


def _prep_inputs(x, wg, w1, b1, w2, b2):
    bf = ml_dtypes.bfloat16
    tokens = np.ascontiguousarray(x.reshape(T, M)).astype(np.float32)
    xT = np.ascontiguousarray(tokens.T)
    xb = tokens.astype(bf)
    wgf = np.ascontiguousarray(wg.astype(np.float32))
    eiota = np.tile(np.arange(E, dtype=np.float32), LT)[None, :].repeat(P, 0)
    triu = np.triu(np.ones((P, P), dtype=np.float32))
    identf = np.eye(P, dtype=np.float32)
    identb = np.eye(P).astype(bf)
    w64 = _make_w64()
    ecl = np.tile(np.arange(E, dtype=np.float32) * CL, LT)[None, :].repeat(P, 0)
    qiota = (np.arange(NB, dtype=np.float32)[None, :] * P
             + np.arange(P, dtype=np.float32)[:, None]).copy()
    siota = np.arange(E, dtype=np.float32)[None, :].repeat(P, 0)
    cpak = np.concatenate(
        [eiota, triu, identf, ecl,
         np.zeros((P, 8), np.float32), qiota, siota], axis=1)
    in_maps = []
    for e in range(E):
        w1e = np.ascontiguousarray(w1[e]).astype(bf)
        w1pk = np.ascontiguousarray(
            w1e.reshape(MC, P, DC, P).transpose(2, 1, 0, 3))
        w2e = np.ascontiguousarray(w2[e]).astype(bf)
        w2pk = np.ascontiguousarray(
            w2e.reshape(DC, P, M).transpose(1, 0, 2))
        tokp1 = (e * TSH + np.arange(TSH, dtype=np.float32)
                 .reshape(LT, P).T + 1.0).copy()
        cpk = cpak.copy()
        cpk[:, 384:392] = tokp1
        meloc = (e * CL + 1 + qiota[:, :NA]).astype(np.int32)
        perc = np.zeros((1, 24), dtype=np.float32)
        for s in range(E):
            perc[0, s] = 1.0 if s == e else 0.0
            perc[0, 8 + s] = s * CL + (NAS + 1 if s == e else 1)
            perc[0, 16 + s] = float(NAS) if s == e else 0.0
        in_maps.append({
            "xTs": np.ascontiguousarray(xT[:, e * TSH:(e + 1) * TSH]),
            "xb": xb, "wg": wgf,
            "w1p": w1pk, "w2p": w2pk,
            "b1v": np.ascontiguousarray(b1[e]).astype(np.float32),
            "b2b": np.tile(np.asarray(b2[e], dtype=np.float32), (P, 1)),
            "cpakd": np.ascontiguousarray(cpk),
            "identb": identb, "w64d": w64,
            "melocd": meloc, "percd": perc,
        })
    return in_maps


def kernel(x, wg, w1, b1, w2, b2, _trace=False):
    if "nc" not in _CACHE:
        _CACHE["nc"] = _build_nc()
    nc = _CACHE["nc"]
    in_maps = _prep_inputs(
        np.asarray(x), np.asarray(wg), np.asarray(w1),
        np.asarray(b1), np.asarray(w2), np.asarray(b2))
    res = run_bass_kernel_spmd(nc, in_maps, list(range(E)), trace=_trace)
    _CACHE["last_results"] = res
    full = np.zeros((T, M), dtype=np.float32)
    for e in range(E):
        full += res.results[e]["out"]
    return full.reshape(B, S, M)
